# revision 12
# baseline (speedup 1.0000x reference)
"""Trainium2 Bass kernel for nn_AttentionBlockSE3 (SE3 graph attention block).

Reference computation (N=20000 nodes, E=320000 edges, C=64 channels, H=8 heads):
  k = to_heads(key_0, key_1)      [E, 8, 32]
  q = to_heads(query_0, query_1)  [N, 8, 32]
  logits = einsum('ehd,ehd->eh', k, q[dst]) / 16
  alpha  = edge_softmax(logits, dst)           (per dst node, per head)
  out_d  = segment_sum(alpha * value_d, dst)   for degree 0 and 1 values
  returns (out_0 [N,64,1], out_1 [N,64,3], prelogits [E,8])

Strategy (edge-parallel across 8 NeuronCores, no collectives needed):
  * Host sorts edges by dst and groups nodes into tiles of 127 (+1 trash slot
    that absorbs padding edges).  Each tile's edges are padded to whole
    128-edge chunks.  Tiles are dealt to the 8 cores so that every core has an
    IDENTICAL compile-time schedule cnt[slot] (SPMD: one program, 8 cores).
  * Per chunk the device builds a one-hot edge->node-slot matrix from the dst
    values (iota + tensor_scalar eq), gathers q via a PE matmul, computes the
    per-head logits on DVE, exp on ACT (softmax without max subtraction -- the
    logits are O(1) so this is numerically safe and matches the reference
    up to fp32 rounding), and scatter-adds the exp-weighted values and the
    softmax denominators into a per-tile PSUM accumulator via a second PE
    matmul.  At the tile boundary the accumulator is divided by the
    denominators and written out.
  * Host inverse-permutes the outputs.
"""

import math
import numpy as np

from concourse import bass, mybir
from concourse.tile import TileContext, ScopedClock
from concourse.bass_utils import run_bass_kernel_spmd

# ----------------------------------------------------------------------------
# Workaround: this walrus build accepts at most ONE sem wait per instruction
# ("Too many sync wait commands").  Split extra waits onto single-wait Drain
# carriers on the same engine, and split the kernel-tail drain the same way.
# ----------------------------------------------------------------------------
_PATCHED = False


def _patch_tile():
    global _PATCHED
    if _PATCHED:
        return
    _PATCHED = True

    orig_add = TileContext._add_instruction

    def _add_instruction(self, inst):
        si = getattr(inst, "sync_info", None)
        if si is not None and si.on_wait and len(si.on_wait) > 1:
            waits = list(si.on_wait)
            for w in waits[:-1]:
                nop = mybir.InstDrain(
                    name=self.nc.get_next_instruction_name(), ins=[], outs=[]
                )
                nop.engine = inst.engine
                nop.sync_info = mybir.SyncInfo(on_wait=[w], on_update=[])
                orig_add(self, nop)
            while len(si.on_wait) > 1:
                si.on_wait.pop(0)
            inst.sync_info = si
        orig_add(self, inst)

    TileContext._add_instruction = _add_instruction

    def _drain_and_barrier(self, tick_clock, wait_clock):
        drain_inst = self.nc.sync.drain()
        wait_clock.add_sem_waits(
            drain_inst.ins, ScopedClock({None: tick_clock.global_clock})
        )
        si = drain_inst.ins.sync_info
        waits = list(si.on_wait) if si and si.on_wait else []
        if len(waits) > 1:
            while len(si.on_wait) > 1:
                si.on_wait.pop()
            drain_inst.ins.sync_info = si
            for w in waits[1:]:
                extra = self.nc.sync.drain()
                esi = extra.ins.sync_info
                if esi is None:
                    esi = mybir.SyncInfo(on_wait=[w], on_update=[])
                else:
                    esi.on_wait.append(w)
                extra.ins.sync_info = esi
        self.nc.all_engine_barrier()
        assert self.sems is not None
        popped = self.nc._tile_sem_poison_stack.pop()
        assert popped is self._sem_poison
        self.nc.clear_and_free_semaphores(list(self.sems.allocated().values()))
        self.nc.all_engine_barrier()

    TileContext._drain_and_barrier = _drain_and_barrier


# ----------------------------------------------------------------------------
# Problem constants
# ----------------------------------------------------------------------------
P = 128          # partitions / edges per chunk / node slots per tile
NPT = 127        # real nodes per tile (slot 127 = trash)
NCORES = 8
H = 8            # heads
FK = 256         # head-major k/q features (8 heads x (8 + 24))
HB = 33          # per-head block in wvw/acc: 8 (v0) + 24 (v1) + 1 (w)
W = H * HB       # 264


# ----------------------------------------------------------------------------
# Host-side preprocessing
# ----------------------------------------------------------------------------
def _plan(dst, N):
    """Sort edges by dst, tile nodes, deal tiles to cores with a shared
    per-slot chunk-count schedule.  Returns the schedule + index arrays."""
    E = dst.shape[0]
    dst = dst.astype(np.int64, copy=False)
    perm = np.argsort(dst, kind="stable")
    dst_s = dst[perm]

    n_tiles = (N + NPT - 1) // NPT
    bound = np.minimum(np.arange(n_tiles + 1) * NPT, N)
    starts = np.searchsorted(dst_s, bound[:-1])
    ends = np.searchsorted(dst_s, bound[1:])
    counts = ends - starts                      # edges per tile
    chunks_t = (counts + P - 1) // P            # chunks per tile

    T = (n_tiles + NCORES - 1) // NCORES        # tile slots per core
    order = np.argsort(-chunks_t, kind="stable")

    core_tiles = np.full((NCORES, T), -1, dtype=np.int64)
    cnt = np.zeros(T, dtype=np.int64)
    for s in range(T):
        grp = order[s * NCORES:(s + 1) * NCORES]
        core_tiles[: len(grp), s] = grp
        cnt[s] = chunks_t[grp].max() if len(grp) else 0

    keep = cnt > 0
    cnt = cnt[keep]
    core_tiles = core_tiles[:, keep]
    T = int(cnt.shape[0])
    n_chunks = int(cnt.sum())
    chunk_off = np.concatenate([[0], np.cumsum(cnt)])[:-1]  # per slot

    return dict(perm=perm, dst_s=dst_s, starts=starts, counts=counts,
                core_tiles=core_tiles, cnt=cnt, chunk_off=chunk_off,
                T=T, n_chunks=n_chunks, E=E, N=N)


def _build_core_inputs(plan, key_0, key_1, query_0, query_1, value_0, value_1):
    """Per-core kv / dstloc / q arrays + row->original-edge index maps."""
    E, N = plan["E"], plan["N"]
    T, n_chunks = plan["T"], plan["n_chunks"]
    cnt, chunk_off = plan["cnt"], plan["chunk_off"]
    core_tiles = plan["core_tiles"]
    perm, dst_s, starts, counts = (plan["perm"], plan["dst_s"],
                                   plan["starts"], plan["counts"])

    import ml_dtypes
    k0 = key_0.reshape(E, H, 8)
    k1 = key_1.reshape(E, H, 24)
    v0 = value_0.reshape(E, H, 8)
    v1 = value_1.reshape(E, H, 24)
    qhm = np.concatenate(
        [query_0.reshape(N, H, 8), query_1.reshape(N, H, 24)], axis=2
    ).reshape(N, FK).astype(np.float32, copy=False)

    rows = n_chunks * P
    ins, metas = [], []
    for c in range(NCORES):
        orig = np.full(rows, -1, dtype=np.int64)
        dl = np.full(rows, NPT, dtype=np.float32)       # pads -> trash slot
        qt = np.zeros((T * P, FK), dtype=np.float32)
        for s in range(T):
            tid = core_tiles[c, s]
            if tid < 0:
                continue
            lo = tid * NPT
            hi = min(lo + NPT, N)
            qt[s * P: s * P + (hi - lo)] = qhm[lo:hi]
            st, ce = starts[tid], counts[tid]
            r0 = chunk_off[s] * P
            orig[r0: r0 + ce] = perm[st: st + ce]
            dl[r0: r0 + ce] = (dst_s[st: st + ce] - lo).astype(np.float32)

        q_hi = qt.astype(ml_dtypes.bfloat16)
        q_lo = (qt - q_hi.astype(np.float32)).astype(ml_dtypes.bfloat16)
        valid = orig >= 0
        oi = orig[valid]
        kv = np.zeros((rows, 512), dtype=np.float32)
        kvk = kv[:, 0:256].reshape(rows, H, 32)
        kvv = kv[:, 256:512].reshape(rows, H, 32)
        kvk[valid, :, 0:8] = k0[oi]
        kvk[valid, :, 8:32] = k1[oi]
        kvv[valid, :, 0:8] = v0[oi]
        kvv[valid, :, 8:32] = v1[oi]

        dstloc = np.ascontiguousarray(dl.reshape(n_chunks, P).T)  # [P, n_chunks]
        ins.append({
            "kv": kv.reshape(n_chunks, P, 512),
            "dl": dstloc,
            "qhi": q_hi.reshape(T, P, FK),
            "qlo": q_lo.reshape(T, P, FK),
            "iota": np.broadcast_to(
                np.arange(P, dtype=np.float32), (P, P)).copy(),
            "ident": np.eye(P, dtype=np.float32),
        })
        metas.append(dict(orig=orig, valid=valid))
    return ins, metas


# ----------------------------------------------------------------------------
# Device program
# ----------------------------------------------------------------------------
def _build_program(T, n_chunks, cnt, scatter="f32", wv_engine="vector"):
    """scatter: "f32" (exact, 4 PE cyc/row) or "f32r" (1 cyc/row, ~2e-4 err).
    The q gather is always bf16 hi+lo (exact to ~1e-5)."""
    _patch_tile()
    nc = bass.Bass("TRN2", target_bir_lowering=False, debug=False,
                   num_devices=NCORES)
    f32 = mybir.dt.float32
    f32r = mybir.dt.float32r
    bf16 = mybir.dt.bfloat16
    s_dt = f32r if scatter == "f32r" else f32
    # one-hot EN dtype: stationary of the scatter matmul.  Values are exactly
    # 0/1 so bf16 would be exact, but keep it the same class as the moving
    # operand to avoid mixed-dtype matmuls.
    en_dt = s_dt

    kv = nc.dram_tensor("kv", [n_chunks, P, 512], f32, kind="ExternalInput").ap()
    dl = nc.dram_tensor("dl", [P, n_chunks], f32, kind="ExternalInput").ap()
    qhi = nc.dram_tensor("qhi", [T, P, FK], bf16, kind="ExternalInput").ap()
    qlo = nc.dram_tensor("qlo", [T, P, FK], bf16, kind="ExternalInput").ap()
    iota = nc.dram_tensor("iota", [P, P], f32, kind="ExternalInput").ap()
    ident = nc.dram_tensor("ident", [P, P], f32, kind="ExternalInput").ap()

    outd = nc.dram_tensor("out", [T, P, 256], f32, kind="ExternalOutput").ap()
    pld = nc.dram_tensor("pl", [P, n_chunks * H], f32, kind="ExternalOutput").ap()

    cnt = [int(x) for x in cnt]
    wv_eng = nc.vector if wv_engine == "vector" else nc.gpsimd
    with TileContext(nc) as tc:
        with (
            tc.tile_pool(name="const", bufs=1) as constp,
            tc.tile_pool(name="qp", bufs=2) as qp,
            tc.tile_pool(name="kvp", bufs=4) as kvp,
            tc.tile_pool(name="ohp", bufs=3) as ohp,
            tc.tile_pool(name="ohnep", bufs=3) as ohnep,
            tc.tile_pool(name="prodp", bufs=2) as prodp,
            tc.tile_pool(name="wvwp", bufs=3) as wvwp,
            tc.tile_pool(name="plp", bufs=2) as plp,
            tc.tile_pool(name="fin", bufs=2) as finp,
            tc.tile_pool(name="psoh", bufs=2, space="PSUM") as psoh,
            tc.tile_pool(name="psqg", bufs=2, space="PSUM") as psqg,
            tc.tile_pool(name="psacc", bufs=2, space="PSUM") as psacc,
        ):
            iota_t = constp.tile([P, P], f32)
            nc.sync.dma_start(out=iota_t[:, :], in_=iota[:, :])
            ident_t = constp.tile([P, P], f32)
            nc.sync.dma_start(out=ident_t[:, :], in_=ident[:, :])
            dl_t = constp.tile([P, n_chunks], f32)
            nc.sync.dma_start(out=dl_t[:, :], in_=dl[:, :])

            ch = 0
            for s in range(T):
                cs = cnt[s]
                qh_t = qp.tile([P, FK], bf16, tag="qh")
                nc.sync.dma_start(out=qh_t[:, :], in_=qhi[s])
                ql_t = qp.tile([P, FK], bf16, tag="ql")
                nc.sync.dma_start(out=ql_t[:, :], in_=qlo[s])
                acc = psacc.tile([P, W], f32, tag="acc")
                pl_t = plp.tile([P, cs * H], f32, tag="pl")

                for c in range(cs):
                    kvt = kvp.tile([P, 512], f32, tag="kv")
                    nc.sync.dma_start(out=kvt[:, :], in_=kv[ch])

                    # one-hot [edge, nodeslot]
                    oh_en = ohp.tile([P, P], en_dt, tag="oh")
                    nc.vector.tensor_scalar(
                        out=oh_en[:, :], in0=iota_t[:, :],
                        scalar1=dl_t[:, ch:ch + 1], scalar2=None,
                        op0=mybir.AluOpType.is_equal)

                    # transpose -> [nodeslot, edge] (bf16 for the gather)
                    oh_ps = psoh.tile([P, P], f32, tag="ohps")
                    nc.tensor.transpose(oh_ps[:, :], oh_en[:, :].bitcast(f32),
                                        ident_t[:, :])
                    oh_ne = ohnep.tile([P, P], bf16, tag="ohne")
                    nc.scalar.copy(out=oh_ne[:, :], in_=oh_ps[:, :])

                    # gather q rows: qg = onehot_ne^T @ (q_hi + q_lo)
                    qg = psqg.tile([P, FK], f32, tag="qg")
                    nc.tensor.matmul(qg[:, :], oh_ne[:, :], qh_t[:, :],
                                     start=True, stop=False)
                    nc.tensor.matmul(qg[:, :], oh_ne[:, :], ql_t[:, :],
                                     start=False, stop=True)

                    # logits -> written straight into the prelog staging tile
                    prod = prodp.tile([P, FK], f32, tag="prod")
                    nc.vector.tensor_tensor(
                        out=prod[:, :], in0=kvt[:, 0:FK], in1=qg[:, :],
                        op=mybir.AluOpType.mult)
                    logits = pl_t[:, c * H:(c + 1) * H]
                    nc.vector.tensor_reduce(
                        out=logits,
                        in_=prod[:, :].rearrange("p (h d) -> p h d", h=H),
                        axis=mybir.AxisListType.X, op=mybir.AluOpType.add)

                    # wvw = [ w*v (256) | w (8) ]
                    wvw = wvwp.tile([P, W], s_dt, tag="wvw")
                    nc.scalar.activation(
                        out=wvw[:, 256:264], in_=logits,
                        func=mybir.ActivationFunctionType.Exp, scale=1.0 / 16.0)
                    w_b = (wvw[:, 256:264].unsqueeze(2)
                           .broadcast_to([P, H, 32]))
                    wv_eng.tensor_tensor(
                        out=wvw[:, 0:256].rearrange("p (h c) -> p h c", h=H),
                        in0=kvt[:, 256:512].rearrange("p (h c) -> p h c", h=H),
                        in1=w_b, op=mybir.AluOpType.mult)

                    nc.tensor.matmul(acc[:, :], oh_en[:, :], wvw[:, :],
                                     start=(c == 0), stop=(c == cs - 1))
                    ch += 1

                den_t = finp.tile([P, H], f32, tag="den")
                nc.vector.tensor_scalar_add(den_t[:, :], acc[:, 256:264], 1e-30)
                rec = finp.tile([P, H], f32, tag="rec")
                nc.vector.reciprocal(out=rec[:, :], in_=den_t[:, :])
                out_t = finp.tile([P, 256], f32, tag="outt")
                rec_b = rec[:, :].unsqueeze(2).broadcast_to([P, H, 32])
                nc.vector.tensor_tensor(
                    out=out_t[:, :].rearrange("p (h c) -> p h c", h=H),
                    in0=acc[:, 0:256].rearrange("p (h c) -> p h c", h=H),
                    in1=rec_b, op=mybir.AluOpType.mult)
                nc.sync.dma_start(out=outd[s], in_=out_t[:, :])
                nc.sync.dma_start(
                    out=pld[:, (ch - cs) * H: ch * H], in_=pl_t[:, :])
    return nc


# ----------------------------------------------------------------------------
# Host-side postprocessing
# ----------------------------------------------------------------------------
def _postprocess(plan, results, metas, dst):
    N, E, T = plan["N"], plan["E"], plan["T"]
    core_tiles = plan["core_tiles"]

    out = np.zeros((N, H, 32), dtype=np.float32)
    prelog = np.zeros((E, H), dtype=np.float32)
    for c in range(NCORES):
        r = results[c]
        o = r["out"].reshape(T, P, H, 32)
        for s in range(T):
            tid = core_tiles[c, s]
            if tid < 0:
                continue
            lo = tid * NPT
            hi = min(lo + NPT, N)
            out[lo:hi] = o[s, : hi - lo]
        m = metas[c]
        valid = m["valid"]
        n_chunks = plan["n_chunks"]
        pl_rows = (np.ascontiguousarray(
            r["pl"].reshape(P, n_chunks, H).transpose(1, 0, 2))
            .reshape(-1, H))
        prelog[m["orig"][valid]] = pl_rows[valid] * np.float32(1.0 / 16.0)

    deg = np.bincount(dst.astype(np.int64), minlength=N)
    out[deg == 0] = 0.0

    out_0 = np.ascontiguousarray(out[:, :, 0:8]).reshape(N, 64, 1)
    out_1 = np.ascontiguousarray(out[:, :, 8:32]).reshape(N, 64, 3)
    return out_0, out_1, prelog


def _ensure_ntff_hook():
    """Register the NTFF profile hook that bass_utils expects under axon.
    The agent image's antenv lacks axon_hooks; synthesize the module and
    wire it to trn_agent_boot's ctypes hook.  Also neuter the cloud
    artifact upload (zero-egress container)."""
    import sys
    import types

    import concourse.bass_utils as bu
    bu.upload_artifacts = lambda tmpdir: "local://" + tmpdir

    try:
        from antenv.axon_hooks import get_axon_ntff_profile_hook  # noqa: F401
        return
    except ImportError:
        pass
    import antenv
    mod = types.ModuleType("antenv.axon_hooks")
    _h = [None]
    mod.set_axon_ntff_profile_hook = lambda h: _h.__setitem__(0, h)
    mod.get_axon_ntff_profile_hook = lambda: _h[0]
    sys.modules["antenv.axon_hooks"] = mod
    antenv.axon_hooks = mod
    from trn_agent_boot.trn_boot import _ntff_profile_via_ctypes
    hook = _ntff_profile_via_ctypes("/opt/axon/libaxon_pjrt.so")
    if hook is not None:
        mod.set_axon_ntff_profile_hook(hook)


# ----------------------------------------------------------------------------
# Entry point
# ----------------------------------------------------------------------------
def kernel(value_0, value_1, key_0, key_1, query_0, query_1, dst,
           _scatter="f32", _wv_engine="vector", _trace=False):
    value_0 = np.asarray(value_0, dtype=np.float32)
    value_1 = np.asarray(value_1, dtype=np.float32)
    key_0 = np.asarray(key_0, dtype=np.float32)
    key_1 = np.asarray(key_1, dtype=np.float32)
    query_0 = np.asarray(query_0, dtype=np.float32)
    query_1 = np.asarray(query_1, dtype=np.float32)
    dst = np.asarray(dst)

    N = query_0.shape[0]
    plan = _plan(dst, N)
    ins, metas = _build_core_inputs(plan, key_0, key_1, query_0, query_1,
                                    value_0, value_1)
    nc = _build_program(plan["T"], plan["n_chunks"], plan["cnt"],
                        scatter=_scatter, wv_engine=_wv_engine)
    if _trace:
        _ensure_ntff_hook()
    res = run_bass_kernel_spmd(nc, ins, list(range(NCORES)), trace=_trace)
    out_0, out_1, prelog = _postprocess(plan, res.results, metas, dst)
    kernel._last_exec_time_ns = res.exec_time_ns
    kernel._last_results = res
    return out_0, out_1, prelog


# revision 17
# speedup vs baseline: 1.4484x; 1.4484x over previous
"""Trainium2 Bass kernel for nn_AttentionBlockSE3 (SE3 graph attention block).

Reference computation (N=20000 nodes, E=320000 edges, C=64 channels, H=8 heads):
  k = to_heads(key_0, key_1)      [E, 8, 32]
  q = to_heads(query_0, query_1)  [N, 8, 32]
  logits = einsum('ehd,ehd->eh', k, q[dst]) / 16
  alpha  = edge_softmax(logits, dst)           (per dst node, per head)
  out_d  = segment_sum(alpha * value_d, dst)   for degree 0 and 1 values
  returns (out_0 [N,64,1], out_1 [N,64,3], prelogits [E,8])

Strategy (edge-parallel across 8 NeuronCores, no collectives needed):
  * Host sorts edges by dst and groups nodes into tiles of 127 (+1 trash slot
    that absorbs padding edges).  Each tile's edges are padded to whole
    128-edge chunks.  Tiles are dealt to the 8 cores so that every core has an
    IDENTICAL compile-time schedule cnt[slot] (SPMD: one program, 8 cores).
  * Per chunk the device builds a one-hot edge->node-slot matrix from the dst
    values (iota + tensor_scalar eq), gathers q via a PE matmul, computes the
    per-head logits on DVE, exp on ACT (softmax without max subtraction -- the
    logits are O(1) so this is numerically safe and matches the reference
    up to fp32 rounding), and scatter-adds the exp-weighted values and the
    softmax denominators into a per-tile PSUM accumulator via a second PE
    matmul.  At the tile boundary the accumulator is divided by the
    denominators and written out.
  * Host inverse-permutes the outputs.
"""

import math
import numpy as np

from concourse import bass, mybir
from concourse.tile import TileContext, ScopedClock
from concourse.bass_utils import run_bass_kernel_spmd

# ----------------------------------------------------------------------------
# Workaround: this walrus build accepts at most ONE sem wait per instruction
# ("Too many sync wait commands").  Split extra waits onto single-wait Drain
# carriers on the same engine, and split the kernel-tail drain the same way.
# ----------------------------------------------------------------------------
_PATCHED = False


def _patch_tile():
    global _PATCHED
    if _PATCHED:
        return
    _PATCHED = True

    orig_add = TileContext._add_instruction

    def _add_instruction(self, inst):
        si = getattr(inst, "sync_info", None)
        if si is not None and si.on_wait and len(si.on_wait) > 1:
            waits = list(si.on_wait)
            for w in waits[:-1]:
                nop = mybir.InstDrain(
                    name=self.nc.get_next_instruction_name(), ins=[], outs=[]
                )
                nop.engine = inst.engine
                nop.sync_info = mybir.SyncInfo(on_wait=[w], on_update=[])
                orig_add(self, nop)
            while len(si.on_wait) > 1:
                si.on_wait.pop(0)
            inst.sync_info = si
        orig_add(self, inst)

    TileContext._add_instruction = _add_instruction

    def _drain_and_barrier(self, tick_clock, wait_clock):
        drain_inst = self.nc.sync.drain()
        wait_clock.add_sem_waits(
            drain_inst.ins, ScopedClock({None: tick_clock.global_clock})
        )
        si = drain_inst.ins.sync_info
        waits = list(si.on_wait) if si and si.on_wait else []
        if len(waits) > 1:
            while len(si.on_wait) > 1:
                si.on_wait.pop()
            drain_inst.ins.sync_info = si
            for w in waits[1:]:
                extra = self.nc.sync.drain()
                esi = extra.ins.sync_info
                if esi is None:
                    esi = mybir.SyncInfo(on_wait=[w], on_update=[])
                else:
                    esi.on_wait.append(w)
                extra.ins.sync_info = esi
        self.nc.all_engine_barrier()
        assert self.sems is not None
        popped = self.nc._tile_sem_poison_stack.pop()
        assert popped is self._sem_poison
        self.nc.clear_and_free_semaphores(list(self.sems.allocated().values()))
        self.nc.all_engine_barrier()

    TileContext._drain_and_barrier = _drain_and_barrier


# ----------------------------------------------------------------------------
# Problem constants
# ----------------------------------------------------------------------------
P = 128          # partitions / edges per chunk / node slots per tile
NPT = 127        # real nodes per tile (slot 127 = trash)
NCORES = 8
H = 8            # heads
FK = 256         # head-major k/q features (8 heads x (8 + 24))
HB = 33          # per-head block in wvw/acc: 8 (v0) + 24 (v1) + 1 (w)
W = H * HB       # 264


# ----------------------------------------------------------------------------
# Host-side preprocessing
# ----------------------------------------------------------------------------
def _plan(dst, N):
    """Sort edges by dst, tile nodes, deal tiles to cores with a shared
    per-slot chunk-count schedule.  Returns the schedule + index arrays."""
    E = dst.shape[0]
    dst = dst.astype(np.int64, copy=False)
    perm = np.argsort(dst, kind="stable")
    dst_s = dst[perm]

    n_tiles = (N + NPT - 1) // NPT
    bound = np.minimum(np.arange(n_tiles + 1) * NPT, N)
    starts = np.searchsorted(dst_s, bound[:-1])
    ends = np.searchsorted(dst_s, bound[1:])
    counts = ends - starts                      # edges per tile
    chunks_t = (counts + P - 1) // P            # chunks per tile
    chunks_t = (chunks_t + 1) // 2 * 2          # even (2-chunk compute macros)

    T = (n_tiles + NCORES - 1) // NCORES        # tile slots per core
    order = np.argsort(-chunks_t, kind="stable")

    core_tiles = np.full((NCORES, T), -1, dtype=np.int64)
    cnt = np.zeros(T, dtype=np.int64)
    for s in range(T):
        grp = order[s * NCORES:(s + 1) * NCORES]
        core_tiles[: len(grp), s] = grp
        cnt[s] = chunks_t[grp].max() if len(grp) else 0

    keep = cnt > 0
    cnt = cnt[keep]
    core_tiles = core_tiles[:, keep]
    T = int(cnt.shape[0])
    n_chunks = int(cnt.sum())
    chunk_off = np.concatenate([[0], np.cumsum(cnt)])[:-1]  # per slot

    return dict(perm=perm, dst_s=dst_s, starts=starts, counts=counts,
                core_tiles=core_tiles, cnt=cnt, chunk_off=chunk_off,
                T=T, n_chunks=n_chunks, E=E, N=N)


def _build_core_inputs(plan, key_0, key_1, query_0, query_1, value_0, value_1):
    """Per-core kv / dstloc / q arrays + row->original-edge index maps."""
    E, N = plan["E"], plan["N"]
    T, n_chunks = plan["T"], plan["n_chunks"]
    cnt, chunk_off = plan["cnt"], plan["chunk_off"]
    core_tiles = plan["core_tiles"]
    perm, dst_s, starts, counts = (plan["perm"], plan["dst_s"],
                                   plan["starts"], plan["counts"])

    import ml_dtypes
    k0 = key_0.reshape(E, H, 8)
    k1 = key_1.reshape(E, H, 24)
    v0 = value_0.reshape(E, H, 8)
    v1 = value_1.reshape(E, H, 24)
    qhm = np.concatenate(
        [query_0.reshape(N, H, 8), query_1.reshape(N, H, 24)], axis=2
    ).reshape(N, FK).astype(np.float32, copy=False)

    rows = n_chunks * P
    ins, metas = [], []
    for c in range(NCORES):
        orig = np.full(rows, -1, dtype=np.int64)
        dl = np.full(rows, NPT, dtype=np.float32)       # pads -> trash slot
        qt = np.zeros((T * P, FK), dtype=np.float32)
        for s in range(T):
            tid = core_tiles[c, s]
            if tid < 0:
                continue
            lo = tid * NPT
            hi = min(lo + NPT, N)
            qt[s * P: s * P + (hi - lo)] = qhm[lo:hi]
            st, ce = starts[tid], counts[tid]
            r0 = chunk_off[s] * P
            orig[r0: r0 + ce] = perm[st: st + ce]
            dl[r0: r0 + ce] = (dst_s[st: st + ce] - lo).astype(np.float32)

        q_hi = qt.astype(ml_dtypes.bfloat16)
        q_lo = (qt - q_hi.astype(np.float32)).astype(ml_dtypes.bfloat16)
        valid = orig >= 0
        oi = orig[valid]
        kv = np.zeros((rows, 512), dtype=np.float32)
        kvk = kv[:, 0:256].reshape(rows, H, 32)
        kvv = kv[:, 256:512].reshape(rows, H, 32)
        kvk[valid, :, 0:8] = k0[oi]
        kvk[valid, :, 8:32] = k1[oi]
        kvv[valid, :, 0:8] = v0[oi]
        kvv[valid, :, 8:32] = v1[oi]

        dstloc = np.ascontiguousarray(dl.reshape(n_chunks, P).T)  # [P, n_chunks]
        kv_pm = np.ascontiguousarray(
            kv.reshape(n_chunks, P, 512).transpose(1, 0, 2)).reshape(P, -1)
        ins.append({
            "kv": kv_pm,
            "dl": dstloc,
            "qhi": q_hi.reshape(T, P, FK),
            "qlo": q_lo.reshape(T, P, FK),
            "iota": np.broadcast_to(
                np.arange(P, dtype=np.float32), (P, P)).copy(),
            "ident": np.eye(P, dtype=np.float32),
        })
        metas.append(dict(orig=orig, valid=valid))
    return ins, metas


# ----------------------------------------------------------------------------
# Device program
# ----------------------------------------------------------------------------
def _build_program(T, n_chunks, cnt, scatter="f32r", wv_engine="gpsimd",
                   kv_group=4):
    """scatter: "f32r" (1 PE cyc/row, outputs ~2e-4) or "f32" (exact, 4x PE).
    The q gather is always bf16 hi+lo (exact to ~1e-5).
    Processes chunks in 2-chunk macros; kv DMA'd in kv_group-chunk batches
    from a partition-major layout (big contiguous descriptors)."""
    _patch_tile()
    nc = bass.Bass("TRN2", target_bir_lowering=False, debug=False,
                   num_devices=NCORES)
    f32 = mybir.dt.float32
    f32r = mybir.dt.float32r
    bf16 = mybir.dt.bfloat16
    s_dt = f32r if scatter == "f32r" else f32

    kv = nc.dram_tensor("kv", [P, n_chunks * 512], f32,
                        kind="ExternalInput").ap()
    dl = nc.dram_tensor("dl", [P, n_chunks], f32, kind="ExternalInput").ap()
    qhi = nc.dram_tensor("qhi", [T, P, FK], bf16, kind="ExternalInput").ap()
    qlo = nc.dram_tensor("qlo", [T, P, FK], bf16, kind="ExternalInput").ap()
    iota = nc.dram_tensor("iota", [P, P], f32, kind="ExternalInput").ap()
    ident = nc.dram_tensor("ident", [P, P], f32, kind="ExternalInput").ap()

    outd = nc.dram_tensor("out", [T, P, 256], f32, kind="ExternalOutput").ap()
    pld = nc.dram_tensor("pl", [P, n_chunks * H], f32, kind="ExternalOutput").ap()

    cnt = [int(x) for x in cnt]
    wv_eng = nc.vector if wv_engine == "vector" else nc.gpsimd
    with TileContext(nc) as tc:
        with (
            tc.tile_pool(name="const", bufs=1) as constp,
            tc.tile_pool(name="qp", bufs=2) as qp,
            tc.tile_pool(name="kvp", bufs=3) as kvp,
            tc.tile_pool(name="ohp", bufs=3) as ohp,
            tc.tile_pool(name="ohnep", bufs=3) as ohnep,
            tc.tile_pool(name="prodp", bufs=3) as prodp,
            tc.tile_pool(name="wvwp", bufs=3) as wvwp,
            tc.tile_pool(name="fin", bufs=2) as finp,
            tc.tile_pool(name="psoh", bufs=3, space="PSUM") as psoh,
            tc.tile_pool(name="psqg", bufs=2, space="PSUM") as psqg,
            tc.tile_pool(name="psacc", bufs=2, space="PSUM") as psacc,
        ):
            iota_t = constp.tile([P, P], f32)
            nc.sync.dma_start(out=iota_t[:, :], in_=iota[:, :])
            ident_t = constp.tile([P, P], f32)
            nc.sync.dma_start(out=ident_t[:, :], in_=ident[:, :])
            dl_t = constp.tile([P, n_chunks], f32)
            nc.sync.dma_start(out=dl_t[:, :], in_=dl[:, :])
            pl_t = constp.tile([P, n_chunks * H], f32)

            ch0 = 0
            for s in range(T):
                cs = cnt[s]
                qh_t = qp.tile([P, FK], bf16, tag="qh")
                nc.sync.dma_start(out=qh_t[:, :], in_=qhi[s])
                ql_t = qp.tile([P, FK], bf16, tag="ql")
                nc.sync.dma_start(out=ql_t[:, :], in_=qlo[s])
                acc = psacc.tile([P, W], f32, tag="acc")

                # batched kv loads (partition-major: one big contiguous
                # span per partition)
                kvts = []
                for g in range(0, cs, kv_group):
                    gn = min(kv_group, cs - g)
                    kvt = kvp.tile([P, kv_group * 512], f32, tag="kv")
                    nc.sync.dma_start(
                        out=kvt[:, 0: gn * 512],
                        in_=kv[:, (ch0 + g) * 512: (ch0 + g + gn) * 512])
                    kvts.append(kvt)

                for m in range(0, cs, 2):
                    ch = ch0 + m
                    kvt = kvts[m // kv_group]
                    koff = (m % kv_group) * 512
                    kv2 = kvt[:, koff: koff + 1024]  # [P, 2*512]

                    # one-hot [edge, nodeslot] per chunk
                    oh2 = ohp.tile([P, 2 * P], s_dt, tag="oh")
                    for j in range(2):
                        nc.vector.tensor_scalar(
                            out=oh2[:, j * P:(j + 1) * P], in0=iota_t[:, :],
                            scalar1=dl_t[:, ch + j: ch + j + 1], scalar2=None,
                            op0=mybir.AluOpType.is_equal)

                    # transpose both -> one PSUM bank -> one ACT copy (bf16)
                    oh_ps = psoh.tile([P, 2 * P], f32, tag="ohps")
                    for j in range(2):
                        nc.tensor.matmul(
                            oh_ps[:, j * P:(j + 1) * P],
                            oh2[:, j * P:(j + 1) * P].bitcast(f32),
                            ident_t[:, :], is_transpose=True,
                            start=True, stop=True, skip_group_check=True)
                    oh_ne = ohnep.tile([P, 2 * P], bf16, tag="ohne")
                    nc.scalar.copy(out=oh_ne[:, :], in_=oh_ps[:, :])

                    # gather q rows for both chunks into one PSUM bank
                    qg2 = psqg.tile([P, 2 * FK], f32, tag="qg")
                    for j in range(2):
                        nc.tensor.matmul(
                            qg2[:, j * FK:(j + 1) * FK],
                            oh_ne[:, j * P:(j + 1) * P], qh_t[:, :],
                            start=True, stop=False, skip_group_check=True)
                        nc.tensor.matmul(
                            qg2[:, j * FK:(j + 1) * FK],
                            oh_ne[:, j * P:(j + 1) * P], ql_t[:, :],
                            start=False, stop=True, skip_group_check=True)

                    # logits for both chunks -> straight into prelog tile
                    prod = prodp.tile([P, 2 * FK], f32, tag="prod")
                    nc.vector.tensor_tensor(
                        out=prod[:, :].rearrange("p (c f) -> p c f", c=2),
                        in0=kv2.rearrange("p (c f) -> p c f", c=2)[:, :, 0:FK],
                        in1=qg2[:, :].rearrange("p (c f) -> p c f", c=2),
                        op=mybir.AluOpType.mult)
                    logits = pl_t[:, ch * H: (ch + 2) * H]  # [P, 16]
                    nc.vector.tensor_reduce(
                        out=logits,
                        in_=prod[:, :].rearrange("p (h d) -> p h d", h=2 * H),
                        axis=mybir.AxisListType.X, op=mybir.AluOpType.add)

                    # wvw = [ w*v (256) | w (8) ] per chunk
                    wvw2 = wvwp.tile([P, 2 * W], s_dt, tag="wvw")
                    wvw2v = wvw2[:, :].rearrange("p (c w) -> p c w", c=2)
                    nc.scalar.activation(
                        out=wvw2v[:, :, 256:264],
                        in_=logits.rearrange("p (c h) -> p c h", c=2),
                        func=mybir.ActivationFunctionType.Exp, scale=1.0 / 16.0)
                    w_b = (wvw2v[:, :, 256:264].unsqueeze(3)
                           .broadcast_to([P, 2, H, 32]))
                    wv_eng.tensor_tensor(
                        out=wvw2v[:, :, 0:256].rearrange(
                            "p c (h f) -> p c h f", h=H),
                        in0=kv2.rearrange("p (c f) -> p c f", c=2)
                        [:, :, 256:512].rearrange("p c (h f) -> p c h f", h=H),
                        in1=w_b, op=mybir.AluOpType.mult)

                    for j in range(2):
                        nc.tensor.matmul(
                            acc[:, :], oh2[:, j * P:(j + 1) * P],
                            wvw2[:, j * W:(j + 1) * W],
                            start=(m + j == 0), stop=(m + j == cs - 1))

                ch0 += cs
                den_t = finp.tile([P, H], f32, tag="den")
                nc.vector.tensor_scalar_add(den_t[:, :], acc[:, 256:264], 1e-30)
                rec = finp.tile([P, H], f32, tag="rec")
                nc.vector.reciprocal(out=rec[:, :], in_=den_t[:, :])
                out_t = finp.tile([P, 256], f32, tag="outt")
                rec_b = rec[:, :].unsqueeze(2).broadcast_to([P, H, 32])
                nc.vector.tensor_tensor(
                    out=out_t[:, :].rearrange("p (h c) -> p h c", h=H),
                    in0=acc[:, 0:256].rearrange("p (h c) -> p h c", h=H),
                    in1=rec_b, op=mybir.AluOpType.mult)
                nc.sync.dma_start(out=outd[s], in_=out_t[:, :])

            nc.sync.dma_start(out=pld[:, :], in_=pl_t[:, :])
    return nc


# ----------------------------------------------------------------------------
# Host-side postprocessing
# ----------------------------------------------------------------------------
def _postprocess(plan, results, metas, dst):
    N, E, T = plan["N"], plan["E"], plan["T"]
    core_tiles = plan["core_tiles"]

    out = np.zeros((N, H, 32), dtype=np.float32)
    prelog = np.zeros((E, H), dtype=np.float32)
    for c in range(NCORES):
        r = results[c]
        o = r["out"].reshape(T, P, H, 32)
        for s in range(T):
            tid = core_tiles[c, s]
            if tid < 0:
                continue
            lo = tid * NPT
            hi = min(lo + NPT, N)
            out[lo:hi] = o[s, : hi - lo]
        m = metas[c]
        valid = m["valid"]
        n_chunks = plan["n_chunks"]
        pl_rows = (np.ascontiguousarray(
            r["pl"].reshape(P, n_chunks, H).transpose(1, 0, 2))
            .reshape(-1, H))
        prelog[m["orig"][valid]] = pl_rows[valid] * np.float32(1.0 / 16.0)

    deg = np.bincount(dst.astype(np.int64), minlength=N)
    out[deg == 0] = 0.0

    out_0 = np.ascontiguousarray(out[:, :, 0:8]).reshape(N, 64, 1)
    out_1 = np.ascontiguousarray(out[:, :, 8:32]).reshape(N, 64, 3)
    return out_0, out_1, prelog


def _ensure_ntff_hook():
    """Register the NTFF profile hook that bass_utils expects under axon.
    The agent image's antenv lacks axon_hooks; synthesize the module and
    wire it to trn_agent_boot's ctypes hook.  Also neuter the cloud
    artifact upload (zero-egress container)."""
    import sys
    import types

    import concourse.bass_utils as bu
    bu.upload_artifacts = lambda tmpdir: "local://" + tmpdir

    try:
        from antenv.axon_hooks import get_axon_ntff_profile_hook  # noqa: F401
        return
    except ImportError:
        pass
    import antenv
    mod = types.ModuleType("antenv.axon_hooks")
    _h = [None]
    mod.set_axon_ntff_profile_hook = lambda h: _h.__setitem__(0, h)
    mod.get_axon_ntff_profile_hook = lambda: _h[0]
    sys.modules["antenv.axon_hooks"] = mod
    antenv.axon_hooks = mod
    from trn_agent_boot.trn_boot import _ntff_profile_via_ctypes
    hook = _ntff_profile_via_ctypes("/opt/axon/libaxon_pjrt.so")
    if hook is not None:
        mod.set_axon_ntff_profile_hook(hook)


# ----------------------------------------------------------------------------
# Entry point
# ----------------------------------------------------------------------------
def kernel(value_0, value_1, key_0, key_1, query_0, query_1, dst,
           _scatter="f32", _wv_engine="vector", _trace=False):
    value_0 = np.asarray(value_0, dtype=np.float32)
    value_1 = np.asarray(value_1, dtype=np.float32)
    key_0 = np.asarray(key_0, dtype=np.float32)
    key_1 = np.asarray(key_1, dtype=np.float32)
    query_0 = np.asarray(query_0, dtype=np.float32)
    query_1 = np.asarray(query_1, dtype=np.float32)
    dst = np.asarray(dst)

    N = query_0.shape[0]
    plan = _plan(dst, N)
    ins, metas = _build_core_inputs(plan, key_0, key_1, query_0, query_1,
                                    value_0, value_1)
    nc = _build_program(plan["T"], plan["n_chunks"], plan["cnt"],
                        scatter=_scatter, wv_engine=_wv_engine)
    if _trace:
        _ensure_ntff_hook()
    res = run_bass_kernel_spmd(nc, ins, list(range(NCORES)), trace=_trace)
    out_0, out_1, prelog = _postprocess(plan, res.results, metas, dst)
    kernel._last_exec_time_ns = res.exec_time_ns
    kernel._last_results = res
    return out_0, out_1, prelog


# revision 18
# speedup vs baseline: 1.6712x; 1.1538x over previous
"""Trainium2 Bass kernel for nn_AttentionBlockSE3 (SE3 graph attention block).

Reference computation (N=20000 nodes, E=320000 edges, C=64 channels, H=8 heads):
  k = to_heads(key_0, key_1)      [E, 8, 32]
  q = to_heads(query_0, query_1)  [N, 8, 32]
  logits = einsum('ehd,ehd->eh', k, q[dst]) / 16
  alpha  = edge_softmax(logits, dst)           (per dst node, per head)
  out_d  = segment_sum(alpha * value_d, dst)   for degree 0 and 1 values
  returns (out_0 [N,64,1], out_1 [N,64,3], prelogits [E,8])

Strategy (edge-parallel across 8 NeuronCores, no collectives needed):
  * Host sorts edges by dst and groups nodes into tiles of 127 (+1 trash slot
    that absorbs padding edges).  Each tile's edges are padded to whole
    128-edge chunks.  Tiles are dealt to the 8 cores so that every core has an
    IDENTICAL compile-time schedule cnt[slot] (SPMD: one program, 8 cores).
  * Per chunk the device builds a one-hot edge->node-slot matrix from the dst
    values (iota + tensor_scalar eq), gathers q via a PE matmul, computes the
    per-head logits on DVE, exp on ACT (softmax without max subtraction -- the
    logits are O(1) so this is numerically safe and matches the reference
    up to fp32 rounding), and scatter-adds the exp-weighted values and the
    softmax denominators into a per-tile PSUM accumulator via a second PE
    matmul.  At the tile boundary the accumulator is divided by the
    denominators and written out.
  * Host inverse-permutes the outputs.
"""

import math
import numpy as np

from concourse import bass, mybir
from concourse.tile import TileContext, ScopedClock
from concourse.bass_utils import run_bass_kernel_spmd

# ----------------------------------------------------------------------------
# Workaround: this walrus build accepts at most ONE sem wait per instruction
# ("Too many sync wait commands").  Split extra waits onto single-wait Drain
# carriers on the same engine, and split the kernel-tail drain the same way.
# ----------------------------------------------------------------------------
_PATCHED = False


def _patch_tile():
    global _PATCHED
    if _PATCHED:
        return
    _PATCHED = True

    orig_add = TileContext._add_instruction

    def _add_instruction(self, inst):
        si = getattr(inst, "sync_info", None)
        if si is not None and si.on_wait and len(si.on_wait) > 1:
            waits = list(si.on_wait)
            for w in waits[:-1]:
                nop = mybir.InstDrain(
                    name=self.nc.get_next_instruction_name(), ins=[], outs=[]
                )
                nop.engine = inst.engine
                nop.sync_info = mybir.SyncInfo(on_wait=[w], on_update=[])
                orig_add(self, nop)
            while len(si.on_wait) > 1:
                si.on_wait.pop(0)
            inst.sync_info = si
        orig_add(self, inst)

    TileContext._add_instruction = _add_instruction

    def _drain_and_barrier(self, tick_clock, wait_clock):
        drain_inst = self.nc.sync.drain()
        wait_clock.add_sem_waits(
            drain_inst.ins, ScopedClock({None: tick_clock.global_clock})
        )
        si = drain_inst.ins.sync_info
        waits = list(si.on_wait) if si and si.on_wait else []
        if len(waits) > 1:
            while len(si.on_wait) > 1:
                si.on_wait.pop()
            drain_inst.ins.sync_info = si
            for w in waits[1:]:
                extra = self.nc.sync.drain()
                esi = extra.ins.sync_info
                if esi is None:
                    esi = mybir.SyncInfo(on_wait=[w], on_update=[])
                else:
                    esi.on_wait.append(w)
                extra.ins.sync_info = esi
        self.nc.all_engine_barrier()
        assert self.sems is not None
        popped = self.nc._tile_sem_poison_stack.pop()
        assert popped is self._sem_poison
        self.nc.clear_and_free_semaphores(list(self.sems.allocated().values()))
        self.nc.all_engine_barrier()

    TileContext._drain_and_barrier = _drain_and_barrier


# ----------------------------------------------------------------------------
# Problem constants
# ----------------------------------------------------------------------------
P = 128          # partitions / edges per chunk / node slots per tile
NPT = 127        # real nodes per tile (slot 127 = trash)
NCORES = 8
H = 8            # heads
FK = 256         # head-major k/q features (8 heads x (8 + 24))
HB = 33          # per-head block in wvw/acc: 8 (v0) + 24 (v1) + 1 (w)
W = H * HB       # 264


# ----------------------------------------------------------------------------
# Host-side preprocessing
# ----------------------------------------------------------------------------
def _plan(dst, N):
    """Sort edges by dst, tile nodes, deal tiles to cores with a shared
    per-slot chunk-count schedule.  Returns the schedule + index arrays."""
    E = dst.shape[0]
    dst = dst.astype(np.int64, copy=False)
    perm = np.argsort(dst, kind="stable")
    dst_s = dst[perm]

    n_tiles = (N + NPT - 1) // NPT
    bound = np.minimum(np.arange(n_tiles + 1) * NPT, N)
    starts = np.searchsorted(dst_s, bound[:-1])
    ends = np.searchsorted(dst_s, bound[1:])
    counts = ends - starts                      # edges per tile
    chunks_t = (counts + P - 1) // P            # chunks per tile
    chunks_t = (chunks_t + 1) // 2 * 2          # even (2-chunk compute macros)

    T = (n_tiles + NCORES - 1) // NCORES        # tile slots per core
    order = np.argsort(-chunks_t, kind="stable")

    core_tiles = np.full((NCORES, T), -1, dtype=np.int64)
    cnt = np.zeros(T, dtype=np.int64)
    for s in range(T):
        grp = order[s * NCORES:(s + 1) * NCORES]
        core_tiles[: len(grp), s] = grp
        cnt[s] = chunks_t[grp].max() if len(grp) else 0

    keep = cnt > 0
    cnt = cnt[keep]
    core_tiles = core_tiles[:, keep]
    T = int(cnt.shape[0])
    n_chunks = int(cnt.sum())
    chunk_off = np.concatenate([[0], np.cumsum(cnt)])[:-1]  # per slot

    return dict(perm=perm, dst_s=dst_s, starts=starts, counts=counts,
                core_tiles=core_tiles, cnt=cnt, chunk_off=chunk_off,
                T=T, n_chunks=n_chunks, E=E, N=N)


def _build_core_inputs(plan, key_0, key_1, query_0, query_1, value_0, value_1):
    """Per-core kv / dstloc / q arrays + row->original-edge index maps."""
    E, N = plan["E"], plan["N"]
    T, n_chunks = plan["T"], plan["n_chunks"]
    cnt, chunk_off = plan["cnt"], plan["chunk_off"]
    core_tiles = plan["core_tiles"]
    perm, dst_s, starts, counts = (plan["perm"], plan["dst_s"],
                                   plan["starts"], plan["counts"])

    import ml_dtypes
    k0 = key_0.reshape(E, H, 8)
    k1 = key_1.reshape(E, H, 24)
    v0 = value_0.reshape(E, H, 8)
    v1 = value_1.reshape(E, H, 24)
    qhm = np.concatenate(
        [query_0.reshape(N, H, 8), query_1.reshape(N, H, 24)], axis=2
    ).reshape(N, FK).astype(np.float32, copy=False)

    rows = n_chunks * P
    ins, metas = [], []
    for c in range(NCORES):
        orig = np.full(rows, -1, dtype=np.int64)
        dl = np.full(rows, NPT, dtype=np.float32)       # pads -> trash slot
        qt = np.zeros((T * P, FK), dtype=np.float32)
        for s in range(T):
            tid = core_tiles[c, s]
            if tid < 0:
                continue
            lo = tid * NPT
            hi = min(lo + NPT, N)
            qt[s * P: s * P + (hi - lo)] = qhm[lo:hi]
            st, ce = starts[tid], counts[tid]
            r0 = chunk_off[s] * P
            orig[r0: r0 + ce] = perm[st: st + ce]
            dl[r0: r0 + ce] = (dst_s[st: st + ce] - lo).astype(np.float32)

        q_hi = qt.astype(ml_dtypes.bfloat16)
        q_lo = (qt - q_hi.astype(np.float32)).astype(ml_dtypes.bfloat16)
        valid = orig >= 0
        oi = orig[valid]
        kv = np.zeros((rows, 512), dtype=np.float32)
        kvk = kv[:, 0:256].reshape(rows, H, 32)
        kvv = kv[:, 256:512].reshape(rows, H, 32)
        kvk[valid, :, 0:8] = k0[oi]
        kvk[valid, :, 8:32] = k1[oi]
        kvv[valid, :, 0:8] = v0[oi]
        kvv[valid, :, 8:32] = v1[oi]

        dstloc = np.ascontiguousarray(dl.reshape(n_chunks, P).T)  # [P, n_chunks]
        kv_pm = np.ascontiguousarray(
            kv.reshape(n_chunks, P, 512).transpose(1, 0, 2)).reshape(P, -1)
        ins.append({
            "kv": kv_pm,
            "dl": dstloc,
            "qhi": q_hi.reshape(T, P, FK),
            "qlo": q_lo.reshape(T, P, FK),
            "iota": np.broadcast_to(
                np.arange(P, dtype=np.float32), (P, P)).copy(),
            "ident": np.eye(P, dtype=np.float32),
        })
        metas.append(dict(orig=orig, valid=valid))
    return ins, metas


# ----------------------------------------------------------------------------
# Device program
# ----------------------------------------------------------------------------
def _build_program(T, n_chunks, cnt, scatter="f32r", wv_engine="gpsimd",
                   kv_group=4):
    """scatter: "f32r" (1 PE cyc/row, outputs ~2e-4) or "f32" (exact, 4x PE).
    The q gather is always bf16 hi+lo (exact to ~1e-5).
    Processes chunks in 2-chunk macros; kv DMA'd in kv_group-chunk batches
    from a partition-major layout (big contiguous descriptors)."""
    _patch_tile()
    nc = bass.Bass("TRN2", target_bir_lowering=False, debug=False,
                   num_devices=NCORES)
    f32 = mybir.dt.float32
    f32r = mybir.dt.float32r
    bf16 = mybir.dt.bfloat16
    s_dt = f32r if scatter == "f32r" else f32

    kv = nc.dram_tensor("kv", [P, n_chunks * 512], f32,
                        kind="ExternalInput").ap()
    dl = nc.dram_tensor("dl", [P, n_chunks], f32, kind="ExternalInput").ap()
    qhi = nc.dram_tensor("qhi", [T, P, FK], bf16, kind="ExternalInput").ap()
    qlo = nc.dram_tensor("qlo", [T, P, FK], bf16, kind="ExternalInput").ap()
    iota = nc.dram_tensor("iota", [P, P], f32, kind="ExternalInput").ap()
    ident = nc.dram_tensor("ident", [P, P], f32, kind="ExternalInput").ap()

    outd = nc.dram_tensor("out", [T, P, 256], f32, kind="ExternalOutput").ap()
    pld = nc.dram_tensor("pl", [P, n_chunks * H], f32, kind="ExternalOutput").ap()

    cnt = [int(x) for x in cnt]
    wv_eng = nc.vector if wv_engine == "vector" else nc.gpsimd
    with TileContext(nc) as tc:
        with (
            tc.tile_pool(name="const", bufs=1) as constp,
            tc.tile_pool(name="qp", bufs=3) as qp,
            tc.tile_pool(name="kvp", bufs=4) as kvp,
            tc.tile_pool(name="ohp", bufs=5) as ohp,
            tc.tile_pool(name="ohnep", bufs=5) as ohnep,
            tc.tile_pool(name="prodp", bufs=4) as prodp,
            tc.tile_pool(name="wvwp", bufs=4) as wvwp,
            tc.tile_pool(name="fin", bufs=2) as finp,
            tc.tile_pool(name="psoh", bufs=3, space="PSUM") as psoh,
            tc.tile_pool(name="psqg", bufs=3, space="PSUM") as psqg,
            tc.tile_pool(name="psacc", bufs=2, space="PSUM") as psacc,
        ):
            iota_t = constp.tile([P, P], f32)
            nc.sync.dma_start(out=iota_t[:, :], in_=iota[:, :])
            ident_t = constp.tile([P, P], f32)
            nc.sync.dma_start(out=ident_t[:, :], in_=ident[:, :])
            dl_t = constp.tile([P, n_chunks], f32)
            nc.sync.dma_start(out=dl_t[:, :], in_=dl[:, :])
            pl_t = constp.tile([P, n_chunks * H], f32)

            ch0 = 0
            for s in range(T):
                cs = cnt[s]
                qh_t = qp.tile([P, FK], bf16, tag="qh")
                nc.sync.dma_start(out=qh_t[:, :], in_=qhi[s])
                ql_t = qp.tile([P, FK], bf16, tag="ql")
                nc.sync.dma_start(out=ql_t[:, :], in_=qlo[s])
                acc = psacc.tile([P, W], f32, tag="acc")

                # batched kv loads (partition-major: one big contiguous
                # span per partition)
                kvts = []
                for g in range(0, cs, kv_group):
                    gn = min(kv_group, cs - g)
                    kvt = kvp.tile([P, kv_group * 512], f32, tag="kv")
                    nc.sync.dma_start(
                        out=kvt[:, 0: gn * 512],
                        in_=kv[:, (ch0 + g) * 512: (ch0 + g + gn) * 512])
                    kvts.append(kvt)

                for m in range(0, cs, 2):
                    ch = ch0 + m
                    kvt = kvts[m // kv_group]
                    koff = (m % kv_group) * 512
                    kv2 = kvt[:, koff: koff + 1024]  # [P, 2*512]

                    # one-hot [edge, nodeslot] per chunk
                    oh2 = ohp.tile([P, 2 * P], s_dt, tag="oh")
                    for j in range(2):
                        nc.vector.tensor_scalar(
                            out=oh2[:, j * P:(j + 1) * P], in0=iota_t[:, :],
                            scalar1=dl_t[:, ch + j: ch + j + 1], scalar2=None,
                            op0=mybir.AluOpType.is_equal)

                    # transpose both -> one PSUM bank -> one ACT copy (bf16)
                    oh_ps = psoh.tile([P, 2 * P], f32, tag="ohps")
                    for j in range(2):
                        nc.tensor.matmul(
                            oh_ps[:, j * P:(j + 1) * P],
                            oh2[:, j * P:(j + 1) * P].bitcast(f32),
                            ident_t[:, :], is_transpose=True,
                            start=True, stop=True, skip_group_check=True)
                    oh_ne = ohnep.tile([P, 2 * P], bf16, tag="ohne")
                    nc.scalar.copy(out=oh_ne[:, :], in_=oh_ps[:, :])

                    # gather q rows for both chunks into one PSUM bank
                    qg2 = psqg.tile([P, 2 * FK], f32, tag="qg")
                    for j in range(2):
                        nc.tensor.matmul(
                            qg2[:, j * FK:(j + 1) * FK],
                            oh_ne[:, j * P:(j + 1) * P], qh_t[:, :],
                            start=True, stop=False, skip_group_check=True)
                        nc.tensor.matmul(
                            qg2[:, j * FK:(j + 1) * FK],
                            oh_ne[:, j * P:(j + 1) * P], ql_t[:, :],
                            start=False, stop=True, skip_group_check=True)

                    # logits for both chunks -> straight into prelog tile
                    prod = prodp.tile([P, 2 * FK], f32, tag="prod")
                    nc.vector.tensor_tensor(
                        out=prod[:, :].rearrange("p (c f) -> p c f", c=2),
                        in0=kv2.rearrange("p (c f) -> p c f", c=2)[:, :, 0:FK],
                        in1=qg2[:, :].rearrange("p (c f) -> p c f", c=2),
                        op=mybir.AluOpType.mult)
                    logits = pl_t[:, ch * H: (ch + 2) * H]  # [P, 16]
                    nc.vector.tensor_reduce(
                        out=logits,
                        in_=prod[:, :].rearrange("p (h d) -> p h d", h=2 * H),
                        axis=mybir.AxisListType.X, op=mybir.AluOpType.add)

                    # wvw = [ w*v (256) | w (8) ] per chunk
                    wvw2 = wvwp.tile([P, 2 * W], s_dt, tag="wvw")
                    wvw2v = wvw2[:, :].rearrange("p (c w) -> p c w", c=2)
                    nc.scalar.activation(
                        out=wvw2v[:, :, 256:264],
                        in_=logits.rearrange("p (c h) -> p c h", c=2),
                        func=mybir.ActivationFunctionType.Exp, scale=1.0 / 16.0)
                    w_b = (wvw2v[:, :, 256:264].unsqueeze(3)
                           .broadcast_to([P, 2, H, 32]))
                    wv_eng.tensor_tensor(
                        out=wvw2v[:, :, 0:256].rearrange(
                            "p c (h f) -> p c h f", h=H),
                        in0=kv2.rearrange("p (c f) -> p c f", c=2)
                        [:, :, 256:512].rearrange("p c (h f) -> p c h f", h=H),
                        in1=w_b, op=mybir.AluOpType.mult)

                    for j in range(2):
                        nc.tensor.matmul(
                            acc[:, :], oh2[:, j * P:(j + 1) * P],
                            wvw2[:, j * W:(j + 1) * W],
                            start=(m + j == 0), stop=(m + j == cs - 1))

                ch0 += cs
                den_t = finp.tile([P, H], f32, tag="den")
                nc.vector.tensor_scalar_add(den_t[:, :], acc[:, 256:264], 1e-30)
                rec = finp.tile([P, H], f32, tag="rec")
                nc.vector.reciprocal(out=rec[:, :], in_=den_t[:, :])
                out_t = finp.tile([P, 256], f32, tag="outt")
                rec_b = rec[:, :].unsqueeze(2).broadcast_to([P, H, 32])
                nc.vector.tensor_tensor(
                    out=out_t[:, :].rearrange("p (h c) -> p h c", h=H),
                    in0=acc[:, 0:256].rearrange("p (h c) -> p h c", h=H),
                    in1=rec_b, op=mybir.AluOpType.mult)
                nc.sync.dma_start(out=outd[s], in_=out_t[:, :])

            nc.sync.dma_start(out=pld[:, :], in_=pl_t[:, :])
    return nc


# ----------------------------------------------------------------------------
# Host-side postprocessing
# ----------------------------------------------------------------------------
def _postprocess(plan, results, metas, dst):
    N, E, T = plan["N"], plan["E"], plan["T"]
    core_tiles = plan["core_tiles"]

    out = np.zeros((N, H, 32), dtype=np.float32)
    prelog = np.zeros((E, H), dtype=np.float32)
    for c in range(NCORES):
        r = results[c]
        o = r["out"].reshape(T, P, H, 32)
        for s in range(T):
            tid = core_tiles[c, s]
            if tid < 0:
                continue
            lo = tid * NPT
            hi = min(lo + NPT, N)
            out[lo:hi] = o[s, : hi - lo]
        m = metas[c]
        valid = m["valid"]
        n_chunks = plan["n_chunks"]
        pl_rows = (np.ascontiguousarray(
            r["pl"].reshape(P, n_chunks, H).transpose(1, 0, 2))
            .reshape(-1, H))
        prelog[m["orig"][valid]] = pl_rows[valid] * np.float32(1.0 / 16.0)

    deg = np.bincount(dst.astype(np.int64), minlength=N)
    out[deg == 0] = 0.0

    out_0 = np.ascontiguousarray(out[:, :, 0:8]).reshape(N, 64, 1)
    out_1 = np.ascontiguousarray(out[:, :, 8:32]).reshape(N, 64, 3)
    return out_0, out_1, prelog


def _ensure_ntff_hook():
    """Register the NTFF profile hook that bass_utils expects under axon.
    The agent image's antenv lacks axon_hooks; synthesize the module and
    wire it to trn_agent_boot's ctypes hook.  Also neuter the cloud
    artifact upload (zero-egress container)."""
    import sys
    import types

    import concourse.bass_utils as bu
    bu.upload_artifacts = lambda tmpdir: "local://" + tmpdir

    try:
        from antenv.axon_hooks import get_axon_ntff_profile_hook  # noqa: F401
        return
    except ImportError:
        pass
    import antenv
    mod = types.ModuleType("antenv.axon_hooks")
    _h = [None]
    mod.set_axon_ntff_profile_hook = lambda h: _h.__setitem__(0, h)
    mod.get_axon_ntff_profile_hook = lambda: _h[0]
    sys.modules["antenv.axon_hooks"] = mod
    antenv.axon_hooks = mod
    from trn_agent_boot.trn_boot import _ntff_profile_via_ctypes
    hook = _ntff_profile_via_ctypes("/opt/axon/libaxon_pjrt.so")
    if hook is not None:
        mod.set_axon_ntff_profile_hook(hook)


# ----------------------------------------------------------------------------
# Entry point
# ----------------------------------------------------------------------------
def kernel(value_0, value_1, key_0, key_1, query_0, query_1, dst,
           _scatter="f32", _wv_engine="vector", _trace=False):
    value_0 = np.asarray(value_0, dtype=np.float32)
    value_1 = np.asarray(value_1, dtype=np.float32)
    key_0 = np.asarray(key_0, dtype=np.float32)
    key_1 = np.asarray(key_1, dtype=np.float32)
    query_0 = np.asarray(query_0, dtype=np.float32)
    query_1 = np.asarray(query_1, dtype=np.float32)
    dst = np.asarray(dst)

    N = query_0.shape[0]
    plan = _plan(dst, N)
    ins, metas = _build_core_inputs(plan, key_0, key_1, query_0, query_1,
                                    value_0, value_1)
    nc = _build_program(plan["T"], plan["n_chunks"], plan["cnt"],
                        scatter=_scatter, wv_engine=_wv_engine)
    if _trace:
        _ensure_ntff_hook()
    res = run_bass_kernel_spmd(nc, ins, list(range(NCORES)), trace=_trace)
    out_0, out_1, prelog = _postprocess(plan, res.results, metas, dst)
    kernel._last_exec_time_ns = res.exec_time_ns
    kernel._last_results = res
    return out_0, out_1, prelog


# revision 19
# speedup vs baseline: 1.7730x; 1.0609x over previous
"""Trainium2 Bass kernel for nn_AttentionBlockSE3 (SE3 graph attention block).

Reference computation (N=20000 nodes, E=320000 edges, C=64 channels, H=8 heads):
  k = to_heads(key_0, key_1)      [E, 8, 32]
  q = to_heads(query_0, query_1)  [N, 8, 32]
  logits = einsum('ehd,ehd->eh', k, q[dst]) / 16
  alpha  = edge_softmax(logits, dst)           (per dst node, per head)
  out_d  = segment_sum(alpha * value_d, dst)   for degree 0 and 1 values
  returns (out_0 [N,64,1], out_1 [N,64,3], prelogits [E,8])

Strategy (edge-parallel across 8 NeuronCores, no collectives needed):
  * Host sorts edges by dst and groups nodes into tiles of 127 (+1 trash slot
    that absorbs padding edges).  Each tile's edges are padded to whole
    128-edge chunks.  Tiles are dealt to the 8 cores so that every core has an
    IDENTICAL compile-time schedule cnt[slot] (SPMD: one program, 8 cores).
  * Per chunk the device builds a one-hot edge->node-slot matrix from the dst
    values (iota + tensor_scalar eq), gathers q via a PE matmul, computes the
    per-head logits on DVE, exp on ACT (softmax without max subtraction -- the
    logits are O(1) so this is numerically safe and matches the reference
    up to fp32 rounding), and scatter-adds the exp-weighted values and the
    softmax denominators into a per-tile PSUM accumulator via a second PE
    matmul.  At the tile boundary the accumulator is divided by the
    denominators and written out.
  * Host inverse-permutes the outputs.
"""

import math
import numpy as np

from concourse import bass, mybir
from concourse.tile import TileContext, ScopedClock
from concourse.bass_utils import run_bass_kernel_spmd

# ----------------------------------------------------------------------------
# Workaround: this walrus build accepts at most ONE sem wait per instruction
# ("Too many sync wait commands").  Split extra waits onto single-wait Drain
# carriers on the same engine, and split the kernel-tail drain the same way.
# ----------------------------------------------------------------------------
_PATCHED = False


def _patch_tile():
    global _PATCHED
    if _PATCHED:
        return
    _PATCHED = True

    orig_add = TileContext._add_instruction

    def _add_instruction(self, inst):
        si = getattr(inst, "sync_info", None)
        if si is not None and si.on_wait and len(si.on_wait) > 1:
            waits = list(si.on_wait)
            for w in waits[:-1]:
                nop = mybir.InstDrain(
                    name=self.nc.get_next_instruction_name(), ins=[], outs=[]
                )
                nop.engine = inst.engine
                nop.sync_info = mybir.SyncInfo(on_wait=[w], on_update=[])
                orig_add(self, nop)
            while len(si.on_wait) > 1:
                si.on_wait.pop(0)
            inst.sync_info = si
        orig_add(self, inst)

    TileContext._add_instruction = _add_instruction

    def _drain_and_barrier(self, tick_clock, wait_clock):
        drain_inst = self.nc.sync.drain()
        wait_clock.add_sem_waits(
            drain_inst.ins, ScopedClock({None: tick_clock.global_clock})
        )
        si = drain_inst.ins.sync_info
        waits = list(si.on_wait) if si and si.on_wait else []
        if len(waits) > 1:
            while len(si.on_wait) > 1:
                si.on_wait.pop()
            drain_inst.ins.sync_info = si
            for w in waits[1:]:
                extra = self.nc.sync.drain()
                esi = extra.ins.sync_info
                if esi is None:
                    esi = mybir.SyncInfo(on_wait=[w], on_update=[])
                else:
                    esi.on_wait.append(w)
                extra.ins.sync_info = esi
        self.nc.all_engine_barrier()
        assert self.sems is not None
        popped = self.nc._tile_sem_poison_stack.pop()
        assert popped is self._sem_poison
        self.nc.clear_and_free_semaphores(list(self.sems.allocated().values()))
        self.nc.all_engine_barrier()

    TileContext._drain_and_barrier = _drain_and_barrier


# ----------------------------------------------------------------------------
# Problem constants
# ----------------------------------------------------------------------------
P = 128          # partitions / edges per chunk / node slots per tile
NPT = 127        # real nodes per tile (slot 127 = trash)
NCORES = 8
H = 8            # heads
FK = 256         # head-major k/q features (8 heads x (8 + 24))
HB = 33          # per-head block in wvw/acc: 8 (v0) + 24 (v1) + 1 (w)
W = H * HB       # 264


# ----------------------------------------------------------------------------
# Host-side preprocessing
# ----------------------------------------------------------------------------
def _plan(dst, N):
    """Sort edges by dst, tile nodes, deal tiles to cores with a shared
    per-slot chunk-count schedule.  Returns the schedule + index arrays."""
    E = dst.shape[0]
    dst = dst.astype(np.int64, copy=False)
    perm = np.argsort(dst, kind="stable")
    dst_s = dst[perm]

    n_tiles = (N + NPT - 1) // NPT
    bound = np.minimum(np.arange(n_tiles + 1) * NPT, N)
    starts = np.searchsorted(dst_s, bound[:-1])
    ends = np.searchsorted(dst_s, bound[1:])
    counts = ends - starts                      # edges per tile
    chunks_t = (counts + P - 1) // P            # chunks per tile
    chunks_t = (chunks_t + 1) // 2 * 2          # even (2-chunk compute macros)

    T = (n_tiles + NCORES - 1) // NCORES        # tile slots per core
    order = np.argsort(-chunks_t, kind="stable")

    core_tiles = np.full((NCORES, T), -1, dtype=np.int64)
    cnt = np.zeros(T, dtype=np.int64)
    for s in range(T):
        grp = order[s * NCORES:(s + 1) * NCORES]
        core_tiles[: len(grp), s] = grp
        cnt[s] = chunks_t[grp].max() if len(grp) else 0

    keep = cnt > 0
    cnt = cnt[keep]
    core_tiles = core_tiles[:, keep]
    T = int(cnt.shape[0])
    n_chunks = int(cnt.sum())
    chunk_off = np.concatenate([[0], np.cumsum(cnt)])[:-1]  # per slot

    return dict(perm=perm, dst_s=dst_s, starts=starts, counts=counts,
                core_tiles=core_tiles, cnt=cnt, chunk_off=chunk_off,
                T=T, n_chunks=n_chunks, E=E, N=N)


def _build_core_inputs(plan, key_0, key_1, query_0, query_1, value_0, value_1):
    """Per-core kv / dstloc / q arrays + row->original-edge index maps."""
    E, N = plan["E"], plan["N"]
    T, n_chunks = plan["T"], plan["n_chunks"]
    cnt, chunk_off = plan["cnt"], plan["chunk_off"]
    core_tiles = plan["core_tiles"]
    perm, dst_s, starts, counts = (plan["perm"], plan["dst_s"],
                                   plan["starts"], plan["counts"])

    import ml_dtypes
    k0 = key_0.reshape(E, H, 8)
    k1 = key_1.reshape(E, H, 24)
    v0 = value_0.reshape(E, H, 8)
    v1 = value_1.reshape(E, H, 24)
    qhm = np.concatenate(
        [query_0.reshape(N, H, 8), query_1.reshape(N, H, 24)], axis=2
    ).reshape(N, FK).astype(np.float32, copy=False)

    rows = n_chunks * P
    ins, metas = [], []
    for c in range(NCORES):
        orig = np.full(rows, -1, dtype=np.int64)
        dl = np.full(rows, NPT, dtype=np.float32)       # pads -> trash slot
        qt = np.zeros((T * P, FK), dtype=np.float32)
        for s in range(T):
            tid = core_tiles[c, s]
            if tid < 0:
                continue
            lo = tid * NPT
            hi = min(lo + NPT, N)
            qt[s * P: s * P + (hi - lo)] = qhm[lo:hi]
            st, ce = starts[tid], counts[tid]
            r0 = chunk_off[s] * P
            orig[r0: r0 + ce] = perm[st: st + ce]
            dl[r0: r0 + ce] = (dst_s[st: st + ce] - lo).astype(np.float32)

        q_hi = qt.astype(ml_dtypes.bfloat16)
        q_lo = (qt - q_hi.astype(np.float32)).astype(ml_dtypes.bfloat16)
        valid = orig >= 0
        oi = orig[valid]
        kv = np.zeros((rows, 512), dtype=np.float32)
        kvk = kv[:, 0:256].reshape(rows, H, 32)
        kvv = kv[:, 256:512].reshape(rows, H, 32)
        kvk[valid, :, 0:8] = k0[oi]
        kvk[valid, :, 8:32] = k1[oi]
        kvv[valid, :, 0:8] = v0[oi]
        kvv[valid, :, 8:32] = v1[oi]

        dstloc = np.ascontiguousarray(dl.reshape(n_chunks, P).T)  # [P, n_chunks]
        kv_pm = np.ascontiguousarray(
            kv.reshape(n_chunks, P, 512).transpose(1, 0, 2)).reshape(P, -1)
        ins.append({
            "kv": kv_pm,
            "dl": dstloc,
            "qhi": q_hi.reshape(T, P, FK),
            "qlo": q_lo.reshape(T, P, FK),
            "iota": np.broadcast_to(
                np.arange(P, dtype=np.float32), (P, P)).copy(),
            "ident": np.eye(P, dtype=np.float32),
        })
        metas.append(dict(orig=orig, valid=valid))
    return ins, metas


# ----------------------------------------------------------------------------
# Device program
# ----------------------------------------------------------------------------
def _build_program(T, n_chunks, cnt, scatter="f32r", wv_engine="gpsimd",
                   kv_group=4):
    """scatter: "f32r" (1 PE cyc/row, outputs ~2e-4) or "f32" (exact, 4x PE).
    The q gather is always bf16 hi+lo (exact to ~1e-5).
    Processes chunks in 2-chunk macros; kv DMA'd in kv_group-chunk batches
    from a partition-major layout (big contiguous descriptors)."""
    _patch_tile()
    nc = bass.Bass("TRN2", target_bir_lowering=False, debug=False,
                   num_devices=NCORES)
    f32 = mybir.dt.float32
    f32r = mybir.dt.float32r
    bf16 = mybir.dt.bfloat16
    s_dt = f32r if scatter == "f32r" else f32

    kv = nc.dram_tensor("kv", [P, n_chunks * 512], f32,
                        kind="ExternalInput").ap()
    dl = nc.dram_tensor("dl", [P, n_chunks], f32, kind="ExternalInput").ap()
    qhi = nc.dram_tensor("qhi", [T, P, FK], bf16, kind="ExternalInput").ap()
    qlo = nc.dram_tensor("qlo", [T, P, FK], bf16, kind="ExternalInput").ap()
    iota = nc.dram_tensor("iota", [P, P], f32, kind="ExternalInput").ap()
    ident = nc.dram_tensor("ident", [P, P], f32, kind="ExternalInput").ap()

    outd = nc.dram_tensor("out", [T, P, 256], f32, kind="ExternalOutput").ap()
    pld = nc.dram_tensor("pl", [P, n_chunks * H], f32, kind="ExternalOutput").ap()

    cnt = [int(x) for x in cnt]
    wv_eng = nc.vector if wv_engine == "vector" else nc.gpsimd
    with TileContext(nc) as tc:
        with (
            tc.tile_pool(name="const", bufs=1) as constp,
            tc.tile_pool(name="qp", bufs=4) as qp,
            tc.tile_pool(name="kvp", bufs=6) as kvp,
            tc.tile_pool(name="ohp", bufs=6) as ohp,
            tc.tile_pool(name="ohnep", bufs=6) as ohnep,
            tc.tile_pool(name="prodp", bufs=5) as prodp,
            tc.tile_pool(name="wvwp", bufs=5) as wvwp,
            tc.tile_pool(name="fin", bufs=2) as finp,
            tc.tile_pool(name="psoh", bufs=3, space="PSUM") as psoh,
            tc.tile_pool(name="psqg", bufs=3, space="PSUM") as psqg,
            tc.tile_pool(name="psacc", bufs=2, space="PSUM") as psacc,
        ):
            iota_t = constp.tile([P, P], f32)
            nc.sync.dma_start(out=iota_t[:, :], in_=iota[:, :])
            ident_t = constp.tile([P, P], f32)
            nc.sync.dma_start(out=ident_t[:, :], in_=ident[:, :])
            dl_t = constp.tile([P, n_chunks], f32)
            nc.sync.dma_start(out=dl_t[:, :], in_=dl[:, :])
            pl_t = constp.tile([P, n_chunks * H], f32)

            ch0 = 0
            for s in range(T):
                cs = cnt[s]
                qh_t = qp.tile([P, FK], bf16, tag="qh")
                nc.sync.dma_start(out=qh_t[:, :], in_=qhi[s])
                ql_t = qp.tile([P, FK], bf16, tag="ql")
                nc.sync.dma_start(out=ql_t[:, :], in_=qlo[s])
                acc = psacc.tile([P, W], f32, tag="acc")

                # batched kv loads (partition-major: one big contiguous
                # span per partition)
                kvts = []
                for g in range(0, cs, kv_group):
                    gn = min(kv_group, cs - g)
                    kvt = kvp.tile([P, kv_group * 512], f32, tag="kv")
                    nc.sync.dma_start(
                        out=kvt[:, 0: gn * 512],
                        in_=kv[:, (ch0 + g) * 512: (ch0 + g + gn) * 512])
                    kvts.append(kvt)

                for m in range(0, cs, 2):
                    ch = ch0 + m
                    kvt = kvts[m // kv_group]
                    koff = (m % kv_group) * 512
                    kv2 = kvt[:, koff: koff + 1024]  # [P, 2*512]

                    # one-hot [edge, nodeslot] per chunk
                    oh2 = ohp.tile([P, 2 * P], s_dt, tag="oh")
                    for j in range(2):
                        nc.vector.tensor_scalar(
                            out=oh2[:, j * P:(j + 1) * P], in0=iota_t[:, :],
                            scalar1=dl_t[:, ch + j: ch + j + 1], scalar2=None,
                            op0=mybir.AluOpType.is_equal)

                    # transpose both -> one PSUM bank -> one ACT copy (bf16)
                    oh_ps = psoh.tile([P, 2 * P], f32, tag="ohps")
                    for j in range(2):
                        nc.tensor.matmul(
                            oh_ps[:, j * P:(j + 1) * P],
                            oh2[:, j * P:(j + 1) * P].bitcast(f32),
                            ident_t[:, :], is_transpose=True,
                            start=True, stop=True, skip_group_check=True)
                    oh_ne = ohnep.tile([P, 2 * P], bf16, tag="ohne")
                    nc.scalar.copy(out=oh_ne[:, :], in_=oh_ps[:, :])

                    # gather q rows for both chunks into one PSUM bank
                    qg2 = psqg.tile([P, 2 * FK], f32, tag="qg")
                    for j in range(2):
                        nc.tensor.matmul(
                            qg2[:, j * FK:(j + 1) * FK],
                            oh_ne[:, j * P:(j + 1) * P], qh_t[:, :],
                            start=True, stop=False, skip_group_check=True)
                        nc.tensor.matmul(
                            qg2[:, j * FK:(j + 1) * FK],
                            oh_ne[:, j * P:(j + 1) * P], ql_t[:, :],
                            start=False, stop=True, skip_group_check=True)

                    # logits for both chunks -> straight into prelog tile
                    prod = prodp.tile([P, 2 * FK], f32, tag="prod")
                    nc.vector.tensor_tensor(
                        out=prod[:, :].rearrange("p (c f) -> p c f", c=2),
                        in0=kv2.rearrange("p (c f) -> p c f", c=2)[:, :, 0:FK],
                        in1=qg2[:, :].rearrange("p (c f) -> p c f", c=2),
                        op=mybir.AluOpType.mult)
                    logits = pl_t[:, ch * H: (ch + 2) * H]  # [P, 16]
                    nc.vector.tensor_reduce(
                        out=logits,
                        in_=prod[:, :].rearrange("p (h d) -> p h d", h=2 * H),
                        axis=mybir.AxisListType.X, op=mybir.AluOpType.add)

                    # wvw = [ w*v (256) | w (8) ] per chunk
                    wvw2 = wvwp.tile([P, 2 * W], s_dt, tag="wvw")
                    wvw2v = wvw2[:, :].rearrange("p (c w) -> p c w", c=2)
                    nc.scalar.activation(
                        out=wvw2v[:, :, 256:264],
                        in_=logits.rearrange("p (c h) -> p c h", c=2),
                        func=mybir.ActivationFunctionType.Exp, scale=1.0 / 16.0)
                    w_b = (wvw2v[:, :, 256:264].unsqueeze(3)
                           .broadcast_to([P, 2, H, 32]))
                    wv_eng.tensor_tensor(
                        out=wvw2v[:, :, 0:256].rearrange(
                            "p c (h f) -> p c h f", h=H),
                        in0=kv2.rearrange("p (c f) -> p c f", c=2)
                        [:, :, 256:512].rearrange("p c (h f) -> p c h f", h=H),
                        in1=w_b, op=mybir.AluOpType.mult)

                    for j in range(2):
                        nc.tensor.matmul(
                            acc[:, :], oh2[:, j * P:(j + 1) * P],
                            wvw2[:, j * W:(j + 1) * W],
                            start=(m + j == 0), stop=(m + j == cs - 1))

                ch0 += cs
                den_t = finp.tile([P, H], f32, tag="den")
                nc.vector.tensor_scalar_add(den_t[:, :], acc[:, 256:264], 1e-30)
                rec = finp.tile([P, H], f32, tag="rec")
                nc.vector.reciprocal(out=rec[:, :], in_=den_t[:, :])
                out_t = finp.tile([P, 256], f32, tag="outt")
                rec_b = rec[:, :].unsqueeze(2).broadcast_to([P, H, 32])
                nc.vector.tensor_tensor(
                    out=out_t[:, :].rearrange("p (h c) -> p h c", h=H),
                    in0=acc[:, 0:256].rearrange("p (h c) -> p h c", h=H),
                    in1=rec_b, op=mybir.AluOpType.mult)
                nc.sync.dma_start(out=outd[s], in_=out_t[:, :])

            nc.sync.dma_start(out=pld[:, :], in_=pl_t[:, :])
    return nc


# ----------------------------------------------------------------------------
# Host-side postprocessing
# ----------------------------------------------------------------------------
def _postprocess(plan, results, metas, dst):
    N, E, T = plan["N"], plan["E"], plan["T"]
    core_tiles = plan["core_tiles"]

    out = np.zeros((N, H, 32), dtype=np.float32)
    prelog = np.zeros((E, H), dtype=np.float32)
    for c in range(NCORES):
        r = results[c]
        o = r["out"].reshape(T, P, H, 32)
        for s in range(T):
            tid = core_tiles[c, s]
            if tid < 0:
                continue
            lo = tid * NPT
            hi = min(lo + NPT, N)
            out[lo:hi] = o[s, : hi - lo]
        m = metas[c]
        valid = m["valid"]
        n_chunks = plan["n_chunks"]
        pl_rows = (np.ascontiguousarray(
            r["pl"].reshape(P, n_chunks, H).transpose(1, 0, 2))
            .reshape(-1, H))
        prelog[m["orig"][valid]] = pl_rows[valid] * np.float32(1.0 / 16.0)

    deg = np.bincount(dst.astype(np.int64), minlength=N)
    out[deg == 0] = 0.0

    out_0 = np.ascontiguousarray(out[:, :, 0:8]).reshape(N, 64, 1)
    out_1 = np.ascontiguousarray(out[:, :, 8:32]).reshape(N, 64, 3)
    return out_0, out_1, prelog


def _ensure_ntff_hook():
    """Register the NTFF profile hook that bass_utils expects under axon.
    The agent image's antenv lacks axon_hooks; synthesize the module and
    wire it to trn_agent_boot's ctypes hook.  Also neuter the cloud
    artifact upload (zero-egress container)."""
    import sys
    import types

    import concourse.bass_utils as bu
    bu.upload_artifacts = lambda tmpdir: "local://" + tmpdir

    try:
        from antenv.axon_hooks import get_axon_ntff_profile_hook  # noqa: F401
        return
    except ImportError:
        pass
    import antenv
    mod = types.ModuleType("antenv.axon_hooks")
    _h = [None]
    mod.set_axon_ntff_profile_hook = lambda h: _h.__setitem__(0, h)
    mod.get_axon_ntff_profile_hook = lambda: _h[0]
    sys.modules["antenv.axon_hooks"] = mod
    antenv.axon_hooks = mod
    from trn_agent_boot.trn_boot import _ntff_profile_via_ctypes
    hook = _ntff_profile_via_ctypes("/opt/axon/libaxon_pjrt.so")
    if hook is not None:
        mod.set_axon_ntff_profile_hook(hook)


# ----------------------------------------------------------------------------
# Entry point
# ----------------------------------------------------------------------------
def kernel(value_0, value_1, key_0, key_1, query_0, query_1, dst,
           _scatter="f32", _wv_engine="vector", _trace=False):
    value_0 = np.asarray(value_0, dtype=np.float32)
    value_1 = np.asarray(value_1, dtype=np.float32)
    key_0 = np.asarray(key_0, dtype=np.float32)
    key_1 = np.asarray(key_1, dtype=np.float32)
    query_0 = np.asarray(query_0, dtype=np.float32)
    query_1 = np.asarray(query_1, dtype=np.float32)
    dst = np.asarray(dst)

    N = query_0.shape[0]
    plan = _plan(dst, N)
    ins, metas = _build_core_inputs(plan, key_0, key_1, query_0, query_1,
                                    value_0, value_1)
    nc = _build_program(plan["T"], plan["n_chunks"], plan["cnt"],
                        scatter=_scatter, wv_engine=_wv_engine)
    if _trace:
        _ensure_ntff_hook()
    res = run_bass_kernel_spmd(nc, ins, list(range(NCORES)), trace=_trace)
    out_0, out_1, prelog = _postprocess(plan, res.results, metas, dst)
    kernel._last_exec_time_ns = res.exec_time_ns
    kernel._last_results = res
    return out_0, out_1, prelog


# revision 20
# speedup vs baseline: 1.7733x; 1.0002x over previous
"""Trainium2 Bass kernel for nn_AttentionBlockSE3 (SE3 graph attention block).

Reference computation (N=20000 nodes, E=320000 edges, C=64 channels, H=8 heads):
  k = to_heads(key_0, key_1)      [E, 8, 32]
  q = to_heads(query_0, query_1)  [N, 8, 32]
  logits = einsum('ehd,ehd->eh', k, q[dst]) / 16
  alpha  = edge_softmax(logits, dst)           (per dst node, per head)
  out_d  = segment_sum(alpha * value_d, dst)   for degree 0 and 1 values
  returns (out_0 [N,64,1], out_1 [N,64,3], prelogits [E,8])

Strategy (edge-parallel across 8 NeuronCores, no collectives needed):
  * Host sorts edges by dst and groups nodes into tiles of 127 (+1 trash slot
    that absorbs padding edges).  Each tile's edges are padded to whole
    128-edge chunks.  Tiles are dealt to the 8 cores so that every core has an
    IDENTICAL compile-time schedule cnt[slot] (SPMD: one program, 8 cores).
  * Per chunk the device builds a one-hot edge->node-slot matrix from the dst
    values (iota + tensor_scalar eq), gathers q via a PE matmul, computes the
    per-head logits on DVE, exp on ACT (softmax without max subtraction -- the
    logits are O(1) so this is numerically safe and matches the reference
    up to fp32 rounding), and scatter-adds the exp-weighted values and the
    softmax denominators into a per-tile PSUM accumulator via a second PE
    matmul.  At the tile boundary the accumulator is divided by the
    denominators and written out.
  * Host inverse-permutes the outputs.
"""

import math
import numpy as np

from concourse import bass, mybir
from concourse.tile import TileContext, ScopedClock
from concourse.bass_utils import run_bass_kernel_spmd

# ----------------------------------------------------------------------------
# Workaround: this walrus build accepts at most ONE sem wait per instruction
# ("Too many sync wait commands").  Split extra waits onto single-wait Drain
# carriers on the same engine, and split the kernel-tail drain the same way.
# ----------------------------------------------------------------------------
_PATCHED = False


def _patch_tile():
    global _PATCHED
    if _PATCHED:
        return
    _PATCHED = True

    orig_add = TileContext._add_instruction

    def _add_instruction(self, inst):
        si = getattr(inst, "sync_info", None)
        if si is not None and si.on_wait and len(si.on_wait) > 1:
            waits = list(si.on_wait)
            for w in waits[:-1]:
                nop = mybir.InstDrain(
                    name=self.nc.get_next_instruction_name(), ins=[], outs=[]
                )
                nop.engine = inst.engine
                nop.sync_info = mybir.SyncInfo(on_wait=[w], on_update=[])
                orig_add(self, nop)
            while len(si.on_wait) > 1:
                si.on_wait.pop(0)
            inst.sync_info = si
        orig_add(self, inst)

    TileContext._add_instruction = _add_instruction

    def _drain_and_barrier(self, tick_clock, wait_clock):
        drain_inst = self.nc.sync.drain()
        wait_clock.add_sem_waits(
            drain_inst.ins, ScopedClock({None: tick_clock.global_clock})
        )
        si = drain_inst.ins.sync_info
        waits = list(si.on_wait) if si and si.on_wait else []
        if len(waits) > 1:
            while len(si.on_wait) > 1:
                si.on_wait.pop()
            drain_inst.ins.sync_info = si
            for w in waits[1:]:
                extra = self.nc.sync.drain()
                esi = extra.ins.sync_info
                if esi is None:
                    esi = mybir.SyncInfo(on_wait=[w], on_update=[])
                else:
                    esi.on_wait.append(w)
                extra.ins.sync_info = esi
        self.nc.all_engine_barrier()
        assert self.sems is not None
        popped = self.nc._tile_sem_poison_stack.pop()
        assert popped is self._sem_poison
        self.nc.clear_and_free_semaphores(list(self.sems.allocated().values()))
        self.nc.all_engine_barrier()

    TileContext._drain_and_barrier = _drain_and_barrier


# ----------------------------------------------------------------------------
# Problem constants
# ----------------------------------------------------------------------------
P = 128          # partitions / edges per chunk / node slots per tile
NPT = 127        # real nodes per tile (slot 127 = trash)
NCORES = 8
H = 8            # heads
FK = 256         # head-major k/q features (8 heads x (8 + 24))
HB = 33          # per-head block in wvw/acc: 8 (v0) + 24 (v1) + 1 (w)
W = H * HB       # 264


# ----------------------------------------------------------------------------
# Host-side preprocessing
# ----------------------------------------------------------------------------
def _plan(dst, N):
    """Sort edges by dst, tile nodes, deal tiles to cores with a shared
    per-slot chunk-count schedule.  Returns the schedule + index arrays."""
    E = dst.shape[0]
    dst = dst.astype(np.int64, copy=False)
    perm = np.argsort(dst, kind="stable")
    dst_s = dst[perm]

    n_tiles = (N + NPT - 1) // NPT
    bound = np.minimum(np.arange(n_tiles + 1) * NPT, N)
    starts = np.searchsorted(dst_s, bound[:-1])
    ends = np.searchsorted(dst_s, bound[1:])
    counts = ends - starts                      # edges per tile
    chunks_t = (counts + P - 1) // P            # chunks per tile
    chunks_t = (chunks_t + 1) // 2 * 2          # even (2-chunk compute macros)

    T = (n_tiles + NCORES - 1) // NCORES        # tile slots per core
    order = np.argsort(-chunks_t, kind="stable")

    core_tiles = np.full((NCORES, T), -1, dtype=np.int64)
    cnt = np.zeros(T, dtype=np.int64)
    for s in range(T):
        grp = order[s * NCORES:(s + 1) * NCORES]
        core_tiles[: len(grp), s] = grp
        cnt[s] = chunks_t[grp].max() if len(grp) else 0

    keep = cnt > 0
    cnt = cnt[keep]
    core_tiles = core_tiles[:, keep]
    T = int(cnt.shape[0])
    n_chunks = int(cnt.sum())
    chunk_off = np.concatenate([[0], np.cumsum(cnt)])[:-1]  # per slot

    return dict(perm=perm, dst_s=dst_s, starts=starts, counts=counts,
                core_tiles=core_tiles, cnt=cnt, chunk_off=chunk_off,
                T=T, n_chunks=n_chunks, E=E, N=N)


def _build_core_inputs(plan, key_0, key_1, query_0, query_1, value_0, value_1):
    """Per-core kv / dstloc / q arrays + row->original-edge index maps."""
    E, N = plan["E"], plan["N"]
    T, n_chunks = plan["T"], plan["n_chunks"]
    cnt, chunk_off = plan["cnt"], plan["chunk_off"]
    core_tiles = plan["core_tiles"]
    perm, dst_s, starts, counts = (plan["perm"], plan["dst_s"],
                                   plan["starts"], plan["counts"])

    import ml_dtypes
    k0 = key_0.reshape(E, H, 8)
    k1 = key_1.reshape(E, H, 24)
    v0 = value_0.reshape(E, H, 8)
    v1 = value_1.reshape(E, H, 24)
    qhm = np.concatenate(
        [query_0.reshape(N, H, 8), query_1.reshape(N, H, 24)], axis=2
    ).reshape(N, FK).astype(np.float32, copy=False)

    rows = n_chunks * P
    ins, metas = [], []
    for c in range(NCORES):
        orig = np.full(rows, -1, dtype=np.int64)
        dl = np.full(rows, NPT, dtype=np.float32)       # pads -> trash slot
        qt = np.zeros((T * P, FK), dtype=np.float32)
        for s in range(T):
            tid = core_tiles[c, s]
            if tid < 0:
                continue
            lo = tid * NPT
            hi = min(lo + NPT, N)
            qt[s * P: s * P + (hi - lo)] = qhm[lo:hi]
            st, ce = starts[tid], counts[tid]
            r0 = chunk_off[s] * P
            orig[r0: r0 + ce] = perm[st: st + ce]
            dl[r0: r0 + ce] = (dst_s[st: st + ce] - lo).astype(np.float32)

        q_hi = qt.astype(ml_dtypes.bfloat16)
        q_lo = (qt - q_hi.astype(np.float32)).astype(ml_dtypes.bfloat16)
        valid = orig >= 0
        oi = orig[valid]
        kv = np.zeros((rows, 512), dtype=np.float32)
        kvk = kv[:, 0:256].reshape(rows, H, 32)
        kvv = kv[:, 256:512].reshape(rows, H, 32)
        kvk[valid, :, 0:8] = k0[oi]
        kvk[valid, :, 8:32] = k1[oi]
        kvv[valid, :, 0:8] = v0[oi]
        kvv[valid, :, 8:32] = v1[oi]

        dstloc = np.ascontiguousarray(dl.reshape(n_chunks, P).T)  # [P, n_chunks]
        kv_pm = np.ascontiguousarray(
            kv.reshape(n_chunks, P, 512).transpose(1, 0, 2)).reshape(P, -1)
        ins.append({
            "kv": kv_pm,
            "dl": dstloc,
            "qhi": q_hi.reshape(T, P, FK),
            "qlo": q_lo.reshape(T, P, FK),
            "iota": np.broadcast_to(
                np.arange(P, dtype=np.float32), (P, P)).copy(),
            "ident": np.eye(P, dtype=np.float32),
        })
        metas.append(dict(orig=orig, valid=valid))
    return ins, metas


# ----------------------------------------------------------------------------
# Device program
# ----------------------------------------------------------------------------
def _build_program(T, n_chunks, cnt, scatter="f32r", wv_engine="gpsimd",
                   kv_group=4):
    """scatter: "f32r" (1 PE cyc/row, outputs ~2e-4) or "f32" (exact, 4x PE).
    The q gather is always bf16 hi+lo (exact to ~1e-5).
    Processes chunks in 2-chunk macros; kv DMA'd in kv_group-chunk batches
    from a partition-major layout (big contiguous descriptors)."""
    _patch_tile()
    nc = bass.Bass("TRN2", target_bir_lowering=False, debug=False,
                   num_devices=NCORES)
    f32 = mybir.dt.float32
    f32r = mybir.dt.float32r
    bf16 = mybir.dt.bfloat16
    s_dt = f32r if scatter == "f32r" else f32

    kv = nc.dram_tensor("kv", [P, n_chunks * 512], f32,
                        kind="ExternalInput").ap()
    dl = nc.dram_tensor("dl", [P, n_chunks], f32, kind="ExternalInput").ap()
    qhi = nc.dram_tensor("qhi", [T, P, FK], bf16, kind="ExternalInput").ap()
    qlo = nc.dram_tensor("qlo", [T, P, FK], bf16, kind="ExternalInput").ap()
    iota = nc.dram_tensor("iota", [P, P], f32, kind="ExternalInput").ap()
    ident = nc.dram_tensor("ident", [P, P], f32, kind="ExternalInput").ap()

    outd = nc.dram_tensor("out", [T, P, 256], f32, kind="ExternalOutput").ap()
    pld = nc.dram_tensor("pl", [P, n_chunks * H], f32, kind="ExternalOutput").ap()

    cnt = [int(x) for x in cnt]
    wv_eng = nc.vector if wv_engine == "vector" else nc.gpsimd
    with TileContext(nc) as tc:
        with (
            tc.tile_pool(name="const", bufs=1) as constp,
            tc.tile_pool(name="qp", bufs=6) as qp,
            tc.tile_pool(name="kvp", bufs=8) as kvp,
            tc.tile_pool(name="ohp", bufs=8) as ohp,
            tc.tile_pool(name="ohnep", bufs=8) as ohnep,
            tc.tile_pool(name="prodp", bufs=6) as prodp,
            tc.tile_pool(name="wvwp", bufs=6) as wvwp,
            tc.tile_pool(name="fin", bufs=3) as finp,
            tc.tile_pool(name="psoh", bufs=3, space="PSUM") as psoh,
            tc.tile_pool(name="psqg", bufs=3, space="PSUM") as psqg,
            tc.tile_pool(name="psacc", bufs=2, space="PSUM") as psacc,
        ):
            iota_t = constp.tile([P, P], f32)
            nc.sync.dma_start(out=iota_t[:, :], in_=iota[:, :])
            ident_t = constp.tile([P, P], f32)
            nc.sync.dma_start(out=ident_t[:, :], in_=ident[:, :])
            dl_t = constp.tile([P, n_chunks], f32)
            nc.sync.dma_start(out=dl_t[:, :], in_=dl[:, :])
            pl_t = constp.tile([P, n_chunks * H], f32)

            ch0 = 0
            for s in range(T):
                cs = cnt[s]
                qh_t = qp.tile([P, FK], bf16, tag="qh")
                nc.sync.dma_start(out=qh_t[:, :], in_=qhi[s])
                ql_t = qp.tile([P, FK], bf16, tag="ql")
                nc.sync.dma_start(out=ql_t[:, :], in_=qlo[s])
                acc = psacc.tile([P, W], f32, tag="acc")

                # batched kv loads (partition-major: one big contiguous
                # span per partition)
                kvts = []
                for g in range(0, cs, kv_group):
                    gn = min(kv_group, cs - g)
                    kvt = kvp.tile([P, kv_group * 512], f32, tag="kv")
                    nc.sync.dma_start(
                        out=kvt[:, 0: gn * 512],
                        in_=kv[:, (ch0 + g) * 512: (ch0 + g + gn) * 512])
                    kvts.append(kvt)

                for m in range(0, cs, 2):
                    ch = ch0 + m
                    kvt = kvts[m // kv_group]
                    koff = (m % kv_group) * 512
                    kv2 = kvt[:, koff: koff + 1024]  # [P, 2*512]

                    # one-hot [edge, nodeslot] per chunk
                    oh2 = ohp.tile([P, 2 * P], s_dt, tag="oh")
                    for j in range(2):
                        nc.vector.tensor_scalar(
                            out=oh2[:, j * P:(j + 1) * P], in0=iota_t[:, :],
                            scalar1=dl_t[:, ch + j: ch + j + 1], scalar2=None,
                            op0=mybir.AluOpType.is_equal)

                    # transpose both -> one PSUM bank -> one ACT copy (bf16)
                    oh_ps = psoh.tile([P, 2 * P], f32, tag="ohps")
                    for j in range(2):
                        nc.tensor.matmul(
                            oh_ps[:, j * P:(j + 1) * P],
                            oh2[:, j * P:(j + 1) * P].bitcast(f32),
                            ident_t[:, :], is_transpose=True,
                            start=True, stop=True, skip_group_check=True)
                    oh_ne = ohnep.tile([P, 2 * P], bf16, tag="ohne")
                    nc.scalar.copy(out=oh_ne[:, :], in_=oh_ps[:, :])

                    # gather q rows for both chunks into one PSUM bank
                    qg2 = psqg.tile([P, 2 * FK], f32, tag="qg")
                    for j in range(2):
                        nc.tensor.matmul(
                            qg2[:, j * FK:(j + 1) * FK],
                            oh_ne[:, j * P:(j + 1) * P], qh_t[:, :],
                            start=True, stop=False, skip_group_check=True)
                        nc.tensor.matmul(
                            qg2[:, j * FK:(j + 1) * FK],
                            oh_ne[:, j * P:(j + 1) * P], ql_t[:, :],
                            start=False, stop=True, skip_group_check=True)

                    # logits for both chunks -> straight into prelog tile
                    prod = prodp.tile([P, 2 * FK], f32, tag="prod")
                    nc.vector.tensor_tensor(
                        out=prod[:, :].rearrange("p (c f) -> p c f", c=2),
                        in0=kv2.rearrange("p (c f) -> p c f", c=2)[:, :, 0:FK],
                        in1=qg2[:, :].rearrange("p (c f) -> p c f", c=2),
                        op=mybir.AluOpType.mult)
                    logits = pl_t[:, ch * H: (ch + 2) * H]  # [P, 16]
                    nc.vector.tensor_reduce(
                        out=logits,
                        in_=prod[:, :].rearrange("p (h d) -> p h d", h=2 * H),
                        axis=mybir.AxisListType.X, op=mybir.AluOpType.add)

                    # wvw = [ w*v (256) | w (8) ] per chunk
                    wvw2 = wvwp.tile([P, 2 * W], s_dt, tag="wvw")
                    wvw2v = wvw2[:, :].rearrange("p (c w) -> p c w", c=2)
                    nc.scalar.activation(
                        out=wvw2v[:, :, 256:264],
                        in_=logits.rearrange("p (c h) -> p c h", c=2),
                        func=mybir.ActivationFunctionType.Exp, scale=1.0 / 16.0)
                    w_b = (wvw2v[:, :, 256:264].unsqueeze(3)
                           .broadcast_to([P, 2, H, 32]))
                    wv_eng.tensor_tensor(
                        out=wvw2v[:, :, 0:256].rearrange(
                            "p c (h f) -> p c h f", h=H),
                        in0=kv2.rearrange("p (c f) -> p c f", c=2)
                        [:, :, 256:512].rearrange("p c (h f) -> p c h f", h=H),
                        in1=w_b, op=mybir.AluOpType.mult)

                    for j in range(2):
                        nc.tensor.matmul(
                            acc[:, :], oh2[:, j * P:(j + 1) * P],
                            wvw2[:, j * W:(j + 1) * W],
                            start=(m + j == 0), stop=(m + j == cs - 1))

                ch0 += cs
                den_t = finp.tile([P, H], f32, tag="den")
                nc.vector.tensor_scalar_add(den_t[:, :], acc[:, 256:264], 1e-30)
                rec = finp.tile([P, H], f32, tag="rec")
                nc.vector.reciprocal(out=rec[:, :], in_=den_t[:, :])
                out_t = finp.tile([P, 256], f32, tag="outt")
                rec_b = rec[:, :].unsqueeze(2).broadcast_to([P, H, 32])
                nc.vector.tensor_tensor(
                    out=out_t[:, :].rearrange("p (h c) -> p h c", h=H),
                    in0=acc[:, 0:256].rearrange("p (h c) -> p h c", h=H),
                    in1=rec_b, op=mybir.AluOpType.mult)
                nc.sync.dma_start(out=outd[s], in_=out_t[:, :])

            nc.sync.dma_start(out=pld[:, :], in_=pl_t[:, :])
    return nc


# ----------------------------------------------------------------------------
# Host-side postprocessing
# ----------------------------------------------------------------------------
def _postprocess(plan, results, metas, dst):
    N, E, T = plan["N"], plan["E"], plan["T"]
    core_tiles = plan["core_tiles"]

    out = np.zeros((N, H, 32), dtype=np.float32)
    prelog = np.zeros((E, H), dtype=np.float32)
    for c in range(NCORES):
        r = results[c]
        o = r["out"].reshape(T, P, H, 32)
        for s in range(T):
            tid = core_tiles[c, s]
            if tid < 0:
                continue
            lo = tid * NPT
            hi = min(lo + NPT, N)
            out[lo:hi] = o[s, : hi - lo]
        m = metas[c]
        valid = m["valid"]
        n_chunks = plan["n_chunks"]
        pl_rows = (np.ascontiguousarray(
            r["pl"].reshape(P, n_chunks, H).transpose(1, 0, 2))
            .reshape(-1, H))
        prelog[m["orig"][valid]] = pl_rows[valid] * np.float32(1.0 / 16.0)

    deg = np.bincount(dst.astype(np.int64), minlength=N)
    out[deg == 0] = 0.0

    out_0 = np.ascontiguousarray(out[:, :, 0:8]).reshape(N, 64, 1)
    out_1 = np.ascontiguousarray(out[:, :, 8:32]).reshape(N, 64, 3)
    return out_0, out_1, prelog


def _ensure_ntff_hook():
    """Register the NTFF profile hook that bass_utils expects under axon.
    The agent image's antenv lacks axon_hooks; synthesize the module and
    wire it to trn_agent_boot's ctypes hook.  Also neuter the cloud
    artifact upload (zero-egress container)."""
    import sys
    import types

    import concourse.bass_utils as bu
    bu.upload_artifacts = lambda tmpdir: "local://" + tmpdir

    try:
        from antenv.axon_hooks import get_axon_ntff_profile_hook  # noqa: F401
        return
    except ImportError:
        pass
    import antenv
    mod = types.ModuleType("antenv.axon_hooks")
    _h = [None]
    mod.set_axon_ntff_profile_hook = lambda h: _h.__setitem__(0, h)
    mod.get_axon_ntff_profile_hook = lambda: _h[0]
    sys.modules["antenv.axon_hooks"] = mod
    antenv.axon_hooks = mod
    from trn_agent_boot.trn_boot import _ntff_profile_via_ctypes
    hook = _ntff_profile_via_ctypes("/opt/axon/libaxon_pjrt.so")
    if hook is not None:
        mod.set_axon_ntff_profile_hook(hook)


# ----------------------------------------------------------------------------
# Entry point
# ----------------------------------------------------------------------------
def kernel(value_0, value_1, key_0, key_1, query_0, query_1, dst,
           _scatter="f32", _wv_engine="vector", _trace=False):
    value_0 = np.asarray(value_0, dtype=np.float32)
    value_1 = np.asarray(value_1, dtype=np.float32)
    key_0 = np.asarray(key_0, dtype=np.float32)
    key_1 = np.asarray(key_1, dtype=np.float32)
    query_0 = np.asarray(query_0, dtype=np.float32)
    query_1 = np.asarray(query_1, dtype=np.float32)
    dst = np.asarray(dst)

    N = query_0.shape[0]
    plan = _plan(dst, N)
    ins, metas = _build_core_inputs(plan, key_0, key_1, query_0, query_1,
                                    value_0, value_1)
    nc = _build_program(plan["T"], plan["n_chunks"], plan["cnt"],
                        scatter=_scatter, wv_engine=_wv_engine)
    if _trace:
        _ensure_ntff_hook()
    res = run_bass_kernel_spmd(nc, ins, list(range(NCORES)), trace=_trace)
    out_0, out_1, prelog = _postprocess(plan, res.results, metas, dst)
    kernel._last_exec_time_ns = res.exec_time_ns
    kernel._last_results = res
    return out_0, out_1, prelog
